# revision 10
# baseline (speedup 1.0000x reference)
"""CSABlock Trainium2 kernel, plan C: act-engine-roofline pipeline.

Core = 2n + h (sample n, image half h). Each core:
  - streams its h-half of feature[n] as bf16 (9.4MB), maxpools over D on DVE
    (bf16 2x mode),
  - exchanges the pooled x chunks with its partner core via fine-grained
    per-512px ReduceScatter (mask trick keeps SPMD code uniform),
  - computes theta (f32r, BN folded into weights host-side) for its 2048
    queries and phi/gT (f32r/bf16) for all 4096 keys locally,
  - runs the 2048x4096 attention in two 1024-query passes; per 128-key chunk:
    scores (PE, f32r) -> exp (Act, the roofline engine: exp is its ONLY job)
    -> weighted accumulate (PE, bf16) -> z accumulation (DVE bf16 trees),
  - tail per pass: z column-sum + 1/z broadcast on PE (no DRAM round trips),
    out conv, residual, store.
"""

import numpy as np
import ml_dtypes

import concourse.bass as bass
import concourse.mybir as mybir
import concourse.tile as tile
from concourse import bacc

F32 = mybir.dt.float32
F32R = mybir.dt.float32r
BF16 = mybir.dt.bfloat16

C = 256
CC = 2            # channel blocks of 128
IC = 128
D = 9
HW = 4096
Q = 2048          # local query/key pixels per core
NM = 4            # streamed pixel chunks of the local half
MPB = Q // NM     # 512 px per chunk
NCH = HW // 128   # 32 key chunks of 128 px over the full image
QP = 1024         # queries per attention pass
NPASS = 2
EXP_BIAS = -30.0
EPS = 1e-5
GROUPS = [[0, 1], [2, 3], [4, 5], [6, 7]]

AF = mybir.ActivationFunctionType
ALU = mybir.AluOpType


def build(nc):
    featd = nc.dram_tensor("feat", [CC, 128, NM, D, MPB], BF16, kind="ExternalInput")
    centerd = nc.dram_tensor("center", [CC, 128, Q], F32R, kind="ExternalInput")
    wthd = nc.dram_tensor("wth", [CC, 128, 128], F32R, kind="ExternalInput")
    wphd = nc.dram_tensor("wph", [CC, 128, 128], BF16, kind="ExternalInput")
    wgd = nc.dram_tensor("wg", [CC, 128, 128], BF16, kind="ExternalInput")
    wwd = nc.dram_tensor("ww", [CC, 128, 128], F32R, kind="ExternalInput")
    bnbd = nc.dram_tensor("bnb", [128, 4], F32, kind="ExternalInput")
    outd = nc.dram_tensor("out", [CC, 128, Q], F32, kind="ExternalOutput")
    # per-m-chunk exchange staging: slot s is the contribution destined for
    # group rank s ([cc0 512px | cc1 512px] bf16). Own slot is mask-zeroed so
    # the 2-core ReduceScatter(add) delivers exactly the partner's x.
    pbd = [nc.dram_tensor(f"pb{m}", [2, 128, 2 * MPB], BF16) for m in range(NM)]
    rsd = [nc.dram_tensor(f"rs{m}", [128, 2 * MPB], BF16) for m in range(NM)]

    with tile.TileContext(nc) as tc:
        with (
            tc.tile_pool(name="persist", bufs=1) as pp,
            tc.tile_pool(name="fstream", bufs=3) as fp,
            tc.tile_pool(name="mp", bufs=6) as mp,
            tc.tile_pool(name="et", bufs=6) as ep,
            tc.tile_pool(name="zt", bufs=10) as zp,
            tc.tile_pool(name="pk", bufs=2) as pk,
            tc.tile_pool(name="ot", bufs=4) as op,
            tc.tile_pool(name="psacc", bufs=1, space="PSUM") as pacc,
            tc.tile_pool(name="pssc", bufs=2, space="PSUM") as pss,
            tc.tile_pool(name="pscv", bufs=2, space="PSUM") as psc,
        ):
            # ---- small loads ----
            center_sb = pp.tile([128, CC, Q], F32R)
            wth = pp.tile([128, CC, 128], F32R)
            wph = pp.tile([128, CC, 128], BF16)
            wg = pp.tile([128, CC, 128], BF16)
            ww = pp.tile([128, CC, 128], F32R)
            bnb = pp.tile([128, 4], F32)
            for cc in range(CC):
                nc.sync.dma_start(out=center_sb[:, cc, :], in_=centerd[cc])
            nc.sync.dma_start(out=wth[:, 0, :], in_=wthd[0])
            nc.sync.dma_start(out=wth[:, 1, :], in_=wthd[1])
            nc.sync.dma_start(out=wph[:, 0, :], in_=wphd[0])
            nc.sync.dma_start(out=wph[:, 1, :], in_=wphd[1])
            nc.sync.dma_start(out=wg[:, 0, :], in_=wgd[0])
            nc.sync.dma_start(out=wg[:, 1, :], in_=wgd[1])
            nc.sync.dma_start(out=ww[:, 0, :], in_=wwd[0])
            nc.sync.dma_start(out=ww[:, 1, :], in_=wwd[1])
            nc.sync.dma_start(out=bnb[:], in_=bnbd[:])

            expb = pp.tile([128, 1], F32)
            nc.gpsimd.memset(expb, EXP_BIAS)
            ones1b = pp.tile([1, 128], BF16)
            nc.gpsimd.memset(ones1b, 1.0)
            ones128b = pp.tile([128, 1], BF16)
            nc.gpsimd.memset(ones128b, 1.0)

            # ---- persistent state ----
            theta = pp.tile([128, Q], F32R)
            xall = pp.tile([128, CC, HW], BF16)   # [ch, cc, px] pooled input
            phi = pp.tile([128, HW], F32R)
            gT = pp.tile([128, NCH, 128], BF16)   # [px-in-chunk, chunk, ch]
            invz = pp.tile([1, QP], F32)
            invzb = pp.tile([1, QP], BF16)

            # ---- theta = relu(wth' @ center + bth) ----
            for qh in range(2):
                ps_t = pss.tile([128, QP], F32, tag="sc")
                for cc in range(CC):
                    for qc in range(2):
                        nc.tensor.matmul(
                            ps_t[:, qc * 512 : (qc + 1) * 512],
                            lhsT=wth[:, cc, :],
                            rhs=center_sb[:, cc, qh * QP + qc * 512 : qh * QP + (qc + 1) * 512],
                            start=(cc == 0),
                            stop=(cc == 1),
                        )
                nc.vector.tensor_scalar(
                    theta[:, qh * QP : (qh + 1) * QP], ps_t,
                    bnb[:, 0:1], 0.0, ALU.add, ALU.max,
                )

            # ---- attention machinery ----
            acc_state = {"acc": None, "ets": [], "quads": [[], []]}

            def conv_chunk(off, chi):
                """phi/gT for 512 px starting at global px `off` (chunk chi*4)."""
                ps_p = psc.tile([128, MPB], F32, tag="cv")
                for cc in range(CC):
                    nc.tensor.matmul(
                        ps_p,
                        lhsT=wph[:, cc, :],
                        rhs=xall[:, cc, off : off + MPB],
                        start=(cc == 0),
                        stop=(cc == 1),
                    )
                nc.vector.tensor_scalar(
                    phi[:, off : off + MPB], ps_p, bnb[:, 1:2], 0.0, ALU.add, ALU.max,
                )
                ps_g = psc.tile([128, MPB], F32, tag="cv")
                for j in range(4):
                    for cc in range(CC):
                        nc.tensor.matmul(
                            ps_g[:, j * 128 : (j + 1) * 128],
                            lhsT=xall[:, cc, off + j * 128 : off + (j + 1) * 128],
                            rhs=wg[:, cc, :],
                            start=(cc == 0),
                            stop=(cc == 1),
                        )
                nc.vector.tensor_copy(gT[:, chi * 4 : chi * 4 + 4, :], ps_g)

            def att_chunk(p, c):
                """pass p, key chunk c: scores -> exp -> weighted + z tree."""
                s_ps = pss.tile([128, QP], F32, tag="sc")
                for qc in range(2):
                    nc.tensor.matmul(
                        s_ps[:, qc * 512 : (qc + 1) * 512],
                        lhsT=phi[:, c * 128 : (c + 1) * 128],
                        rhs=theta[:, p * QP + qc * 512 : p * QP + (qc + 1) * 512],
                        start=True,
                        stop=True,
                    )
                et = ep.tile([128, QP], BF16, tag="et")
                nc.scalar.activation(et, s_ps, AF.Exp, bias=expb[:])
                acc = acc_state["acc"]
                for qc in range(2):
                    nc.tensor.matmul(
                        acc[:, qc * 512 : (qc + 1) * 512],
                        lhsT=gT[:, c, :],
                        rhs=et[:, qc * 512 : (qc + 1) * 512],
                        start=(c == 0),
                        stop=(c == NCH - 1),
                    )
                ets = acc_state["ets"]
                ets.append(et)
                if len(ets) == 2:
                    pr = zp.tile([128, QP], BF16, tag="pair", bufs=4)
                    nc.vector.tensor_add(pr, ets[0], ets[1])
                    acc_state["ets"] = []
                    prs = acc_state["quads"][0]
                    prs.append(pr)
                    if len(prs) == 2:
                        qd = zp.tile([128, QP], BF16, tag="quad", bufs=10)
                        nc.vector.tensor_add(qd, prs[0], prs[1])
                        acc_state["quads"][0] = []
                        acc_state["quads"][1].append(qd)

            def pass_tail(p):
                """z reduce + normalize + out conv + residual + store."""
                quads = acc_state["quads"][1]
                assert len(quads) == 8
                acc_state["quads"][1] = []
                s0 = zp.tile([128, QP], BF16, tag="pair", bufs=4)
                s1 = zp.tile([128, QP], BF16, tag="pair", bufs=4)
                s2 = zp.tile([128, QP], BF16, tag="pair", bufs=4)
                s3 = zp.tile([128, QP], BF16, tag="pair", bufs=4)
                nc.vector.tensor_add(s0, quads[0], quads[1])
                nc.vector.tensor_add(s1, quads[2], quads[3])
                nc.vector.tensor_add(s2, quads[4], quads[5])
                nc.vector.tensor_add(s3, quads[6], quads[7])
                t0 = zp.tile([128, QP], BF16, tag="quad", bufs=10)
                t1 = zp.tile([128, QP], BF16, tag="quad", bufs=10)
                nc.vector.tensor_add(t0, s0, s1)
                nc.vector.tensor_add(t1, s2, s3)
                zfin = zp.tile([128, QP], BF16, tag="zfin", bufs=2)
                nc.vector.tensor_add(zfin, t0, t1)

                # z row sums + reciprocal + broadcast (all on-chip)
                for qc in range(2):
                    zrow = psc.tile([128, MPB], F32, tag="cv")
                    nc.tensor.matmul(
                        zrow[0:1, :],
                        lhsT=ones128b[:, 0:1],
                        rhs=zfin[:, qc * 512 : (qc + 1) * 512],
                        start=True,
                        stop=True,
                    )
                    nc.vector.reciprocal(
                        invz[:, qc * 512 : (qc + 1) * 512], zrow[0:1, :]
                    )
                nc.gpsimd.tensor_copy(invzb, invz)
                bps = pss.tile([128, QP], F32, tag="sc")
                for qc in range(2):
                    nc.tensor.matmul(
                        bps[:, qc * 512 : (qc + 1) * 512],
                        lhsT=ones1b[0:1, :],
                        rhs=invzb[:, qc * 512 : (qc + 1) * 512],
                        start=True,
                        stop=True,
                    )
                bpsb = op.tile([128, QP], BF16, tag="bps", bufs=2)
                nc.vector.tensor_copy(bpsb, bps)
                wsb = pp.tile([128, NPASS, QP], F32R)
                nc.vector.tensor_mul(wsb[:, p, :], acc_state["acc"], bpsb)
                for oc in range(CC):
                    for qc in range(2):
                        pso = psc.tile([128, MPB], F32, tag="cv")
                        nc.tensor.matmul(
                            pso,
                            lhsT=ww[:, oc, :],
                            rhs=wsb[:, p, qc * 512 : (qc + 1) * 512],
                            start=True,
                            stop=True,
                        )
                        osb = op.tile([128, MPB], F32, tag="ot")
                        nc.vector.tensor_add(
                            osb, pso,
                            center_sb[:, oc, p * QP + qc * 512 : p * QP + (qc + 1) * 512],
                        )
                        nc.sync.dma_start(
                            out=outd[oc][:, p * QP + qc * 512 : p * QP + (qc + 1) * 512],
                            in_=osb,
                        )

            # ---- m-loop: stream local feature, maxpool, exchange, conv,
            # and the local 16 chunks of pass 0 ----
            acc_state["acc"] = pacc.tile([128, QP], F32, tag="acc", name="acc")
            for m in range(NM):
                for cc in range(CC):
                    ft = fp.tile([128, D, MPB], BF16, tag="ft")
                    nc.sync.dma_start(out=ft[:], in_=featd[cc, :, m])
                    t_a = mp.tile([128, MPB], BF16, tag="mp")
                    t_b = mp.tile([128, MPB], BF16, tag="mp")
                    t_c = mp.tile([128, MPB], BF16, tag="mp")
                    t_d = mp.tile([128, MPB], BF16, tag="mp")
                    nc.vector.tensor_max(t_a, ft[:, 0, :], ft[:, 1, :])
                    nc.vector.tensor_max(t_b, ft[:, 2, :], ft[:, 3, :])
                    nc.vector.tensor_max(t_c, ft[:, 4, :], ft[:, 5, :])
                    nc.vector.tensor_max(t_d, ft[:, 6, :], ft[:, 7, :])
                    nc.vector.tensor_max(t_a, t_a, t_b)
                    nc.vector.tensor_max(t_c, t_c, t_d)
                    nc.vector.tensor_max(t_a, t_a, t_c)
                    nc.vector.tensor_max(
                        xall[:, cc, m * MPB : (m + 1) * MPB], t_a, ft[:, 8, :]
                    )
                # exchange this chunk with the partner (own slot mask-zeroed)
                pkm = pk.tile([128, 2, 2 * MPB], BF16, tag="pk")
                for s in range(2):
                    for cc in range(CC):
                        nc.gpsimd.tensor_scalar(
                            pkm[:, s, cc * MPB : (cc + 1) * MPB],
                            xall[:, cc, m * MPB : (m + 1) * MPB],
                            bnb[:, 2 + s : 3 + s], None, ALU.mult,
                        )
                    nc.gpsimd.dma_start(out=pbd[m][s], in_=pkm[:, s, :])
                nc.gpsimd.collective_compute(
                    "ReduceScatter", ALU.add, replica_groups=GROUPS,
                    ins=[pbd[m].ap().opt()], outs=[rsd[m].ap().opt()],
                )
                for cc in range(CC):
                    nc.gpsimd.dma_start(
                        out=xall[:, cc, Q + m * MPB : Q + (m + 1) * MPB],
                        in_=rsd[m][:, cc * MPB : (cc + 1) * MPB],
                    )
                conv_chunk(m * MPB, m)
                for c in range(4 * m, 4 * m + 4):
                    att_chunk(0, c)

            # ---- remote chunks of pass 0 (wait on the exchange) ----
            for rm in range(NM):
                conv_chunk(Q + rm * MPB, NM + rm)
                for c in range(16 + 4 * rm, 16 + 4 * rm + 4):
                    att_chunk(0, c)
            pass_tail(0)

            # ---- pass 1: everything resident ----
            acc_state["acc"] = pacc.tile([128, QP], F32, tag="acc", name="acc")
            for c in range(NCH):
                att_chunk(1, c)
            pass_tail(1)


def shard_inputs(inputs):
    f32 = np.float32
    bf16 = ml_dtypes.bfloat16
    feature = np.asarray(inputs["feature"], dtype=f32)
    w_theta = np.asarray(inputs["w_theta"], dtype=f32)
    w_phi = np.asarray(inputs["w_phi"], dtype=f32)
    w_g = np.asarray(inputs["w_g"], dtype=f32)
    w_w = np.asarray(inputs["w_w"], dtype=f32)

    # fold BN (inference) into the conv weights: y = W'x + b'
    sc_th = np.asarray(inputs["bn_theta_gamma"], f32) / np.sqrt(
        np.asarray(inputs["bn_theta_var"], f32) + EPS
    )
    b_th = np.asarray(inputs["bn_theta_beta"], f32) - np.asarray(
        inputs["bn_theta_mean"], f32
    ) * sc_th
    sc_ph = np.asarray(inputs["bn_phi_gamma"], f32) / np.sqrt(
        np.asarray(inputs["bn_phi_var"], f32) + EPS
    )
    b_ph = np.asarray(inputs["bn_phi_beta"], f32) - np.asarray(
        inputs["bn_phi_mean"], f32
    ) * sc_ph

    wth = np.ascontiguousarray((w_theta * sc_th[:, None]).T.reshape(2, 128, 128))
    wph = np.ascontiguousarray(
        (w_phi * sc_ph[:, None]).T.reshape(2, 128, 128)
    ).astype(bf16)
    wgT = np.ascontiguousarray(w_g.T.reshape(2, 128, 128)).astype(bf16)
    wwT = np.ascontiguousarray(w_w.T.reshape(128, 2, 128).transpose(1, 0, 2))

    in_maps = []
    for core in range(8):
        n, h = core // 2, core % 2
        fh = feature[n].reshape(2, 128, D, HW)[:, :, :, h * Q : (h + 1) * Q]
        feat = np.ascontiguousarray(
            fh.reshape(2, 128, D, NM, MPB).transpose(0, 1, 3, 2, 4).astype(bf16)
        )
        center = np.ascontiguousarray(
            feature[n][:, D // 2 + 1].reshape(256, HW)[:, h * Q : (h + 1) * Q]
            .reshape(2, 128, Q)
        )
        bnb = np.zeros((128, 4), dtype=f32)
        bnb[:, 0] = b_th
        bnb[:, 1] = b_ph
        bnb[:, 2 + (1 - h)] = 1.0
        in_maps.append(
            dict(feat=feat, center=center, wth=wth, wph=wph, wg=wgT,
                 ww=wwT, bnb=bnb)
        )
    return in_maps


def unshard_output(results, N=4):
    out = np.empty((N, 256, 64, 64), dtype=np.float32)
    flat = out.reshape(N, 256, HW)
    for core in range(8):
        n, qh = core // 2, core % 2
        flat[n][:, qh * Q : (qh + 1) * Q] = results[core]["out"].reshape(256, Q)
    return out


def make_nc():
    nc = bacc.Bacc("TRN2", target_bir_lowering=False, debug=False, num_devices=8)
    build(nc)
    nc.compile()
    return nc


# ---------------------------------------------------------------------------
# Public entrypoint: full (unsharded) inputs -> full output, running the Bass
# kernel SPMD across the 8 NeuronCores.
# ---------------------------------------------------------------------------
from concourse.bass_utils import run_bass_kernel_spmd

_NC_CACHE = []


def _get_nc():
    if not _NC_CACHE:
        _NC_CACHE.append(make_nc())
    return _NC_CACHE[0]


def kernel(**inputs):
    nc = _get_nc()
    in_maps = shard_inputs(inputs)
    res = run_bass_kernel_spmd(nc, in_maps, list(range(8)))
    return unshard_output(res.results)


# revision 17
# speedup vs baseline: 1.2652x; 1.2652x over previous
"""CSABlock Trainium2 kernel, plan C: act-engine-roofline pipeline.

Core = 2n + h (sample n, image half h). Each core:
  - streams its h-half of feature[n] as bf16 (9.4MB), maxpools over D on DVE
    (bf16 2x mode),
  - exchanges the pooled x chunks with its partner core via fine-grained
    per-512px ReduceScatter (mask trick keeps SPMD code uniform),
  - computes theta (f32r, BN folded into weights host-side) for its 2048
    queries and phi/gT (f32r/bf16) for all 4096 keys locally,
  - runs the 2048x4096 attention in two 1024-query passes; per 128-key chunk:
    scores (PE, f32r) -> exp (Act, the roofline engine: exp is its ONLY job)
    -> weighted accumulate (PE, bf16) -> z accumulation (DVE bf16 trees),
  - tail per pass: z column-sum + 1/z broadcast on PE (no DRAM round trips),
    out conv, residual, store.
"""

import numpy as np
import ml_dtypes

import concourse.bass as bass
import concourse.mybir as mybir
import concourse.tile as tile
from concourse import bacc

F32 = mybir.dt.float32
F32R = mybir.dt.float32r
BF16 = mybir.dt.bfloat16

C = 256
CC = 2            # channel blocks of 128
IC = 128
D = 9
HW = 4096
Q = 2048          # local query/key pixels per core
NM = 4            # streamed pixel chunks of the local half
MPB = Q // NM     # 512 px per chunk
NCH = HW // 128   # 32 key chunks of 128 px over the full image
QP = 1024         # queries per attention pass
NPASS = 2
EXP_BIAS = -30.0
EPS = 1e-5
GROUPS = [[0, 1], [2, 3], [4, 5], [6, 7]]

AF = mybir.ActivationFunctionType
ALU = mybir.AluOpType


def build(nc):
    featd = nc.dram_tensor("feat", [CC, 128, NM, D, MPB], BF16, kind="ExternalInput")
    centerd = nc.dram_tensor("center", [CC, 128, Q], F32R, kind="ExternalInput")
    wthd = nc.dram_tensor("wth", [CC, 128, 128], F32R, kind="ExternalInput")
    wphd = nc.dram_tensor("wph", [CC, 128, 128], BF16, kind="ExternalInput")
    wgd = nc.dram_tensor("wg", [CC, 128, 128], BF16, kind="ExternalInput")
    wwd = nc.dram_tensor("ww", [CC, 128, 128], F32R, kind="ExternalInput")
    bnbd = nc.dram_tensor("bnb", [128, 2], F32, kind="ExternalInput")
    outd = nc.dram_tensor("out", [CC, 128, Q], F32, kind="ExternalOutput")
    # per-m-chunk exchange staging: each core contributes its pooled x chunk
    # ([cc0 512px | cc1 512px] bf16); a 2-core AllGather returns both cores'
    # chunks in rank order. Both slots are read back into xall, so the global
    # key order is [core0's 2048 px | core1's 2048 px] on BOTH cores —
    # SPMD-uniform, and attention is key-permutation-invariant. Queries stay
    # local (center/theta are per-core data).
    pbd = [nc.dram_tensor(f"pb{m}", [128, 2 * MPB], BF16) for m in range(NM)]
    agd = [nc.dram_tensor(f"ag{m}", [2, 128, 2 * MPB], BF16) for m in range(NM)]

    with tile.TileContext(nc) as tc:
        with (
            tc.tile_pool(name="persist", bufs=1) as pp,
            tc.tile_pool(name="fstream", bufs=3) as fp,
            tc.tile_pool(name="mp", bufs=6) as mp,
            tc.tile_pool(name="et", bufs=6) as ep,
            tc.tile_pool(name="zt", bufs=10) as zp,
            tc.tile_pool(name="pk", bufs=2) as pk,
            tc.tile_pool(name="ot", bufs=4) as op,
            tc.tile_pool(name="psacc", bufs=1, space="PSUM") as pacc,
            tc.tile_pool(name="pssc", bufs=2, space="PSUM") as pss,
            tc.tile_pool(name="pscv", bufs=2, space="PSUM") as psc,
        ):
            # ---- small loads ----
            center_sb = pp.tile([128, CC, Q], F32R)
            wth = pp.tile([128, CC, 128], F32R)
            wph = pp.tile([128, CC, 128], BF16)
            wg = pp.tile([128, CC, 128], BF16)
            ww = pp.tile([128, CC, 128], F32R)
            bnb = pp.tile([128, 2], F32)
            for cc in range(CC):
                nc.sync.dma_start(out=center_sb[:, cc, :], in_=centerd[cc])
            nc.sync.dma_start(out=wth[:, 0, :], in_=wthd[0])
            nc.sync.dma_start(out=wth[:, 1, :], in_=wthd[1])
            nc.sync.dma_start(out=wph[:, 0, :], in_=wphd[0])
            nc.sync.dma_start(out=wph[:, 1, :], in_=wphd[1])
            nc.sync.dma_start(out=wg[:, 0, :], in_=wgd[0])
            nc.sync.dma_start(out=wg[:, 1, :], in_=wgd[1])
            nc.sync.dma_start(out=ww[:, 0, :], in_=wwd[0])
            nc.sync.dma_start(out=ww[:, 1, :], in_=wwd[1])
            nc.sync.dma_start(out=bnb[:], in_=bnbd[:])

            expb = pp.tile([128, 1], F32)
            nc.gpsimd.memset(expb, EXP_BIAS)
            ones1b = pp.tile([1, 128], BF16)
            nc.gpsimd.memset(ones1b, 1.0)
            ones128b = pp.tile([128, 1], BF16)
            nc.gpsimd.memset(ones128b, 1.0)

            # ---- persistent state ----
            theta = pp.tile([128, Q], F32R)
            xall = pp.tile([128, CC, HW], BF16)   # [ch, cc, px] pooled input
            phi = pp.tile([128, HW], F32R)
            gT = pp.tile([128, NCH, 128], BF16)   # [px-in-chunk, chunk, ch]
            zrowb = pp.tile([1, QP], BF16)

            # ---- theta = relu(wth' @ center + bth) ----
            for qh in range(2):
                ps_t = pss.tile([128, QP], F32, tag="sc")
                for cc in range(CC):
                    for qc in range(2):
                        nc.tensor.matmul(
                            ps_t[:, qc * 512 : (qc + 1) * 512],
                            lhsT=wth[:, cc, :],
                            rhs=center_sb[:, cc, qh * QP + qc * 512 : qh * QP + (qc + 1) * 512],
                            start=(cc == 0),
                            stop=(cc == 1),
                        )
                nc.vector.tensor_scalar(
                    theta[:, qh * QP : (qh + 1) * QP], ps_t,
                    bnb[:, 0:1], 0.0, ALU.add, ALU.max,
                )

            # ---- attention machinery ----
            acc_state = {"acc": None, "ets": [], "quads": [[], []]}

            def conv_chunk(off, chi):
                """phi/gT for 512 px starting at global px `off` (chunk chi*4)."""
                ps_p = psc.tile([128, MPB], F32, tag="cv")
                for cc in range(CC):
                    nc.tensor.matmul(
                        ps_p,
                        lhsT=wph[:, cc, :],
                        rhs=xall[:, cc, off : off + MPB],
                        start=(cc == 0),
                        stop=(cc == 1),
                    )
                nc.vector.tensor_scalar(
                    phi[:, off : off + MPB], ps_p, bnb[:, 1:2], 0.0, ALU.add, ALU.max,
                )
                ps_g = psc.tile([128, MPB], F32, tag="cv")
                for j in range(4):
                    for cc in range(CC):
                        nc.tensor.matmul(
                            ps_g[:, j * 128 : (j + 1) * 128],
                            lhsT=xall[:, cc, off + j * 128 : off + (j + 1) * 128],
                            rhs=wg[:, cc, :],
                            start=(cc == 0),
                            stop=(cc == 1),
                        )
                nc.vector.tensor_copy(gT[:, chi * 4 : chi * 4 + 4, :], ps_g)

            def att_chunk(p, c, first, last):
                """pass p, key chunk c: scores -> exp -> weighted + z tree."""
                s_ps = pss.tile([128, QP], F32, tag="sc")
                for qc in range(2):
                    nc.tensor.matmul(
                        s_ps[:, qc * 512 : (qc + 1) * 512],
                        lhsT=phi[:, c * 128 : (c + 1) * 128],
                        rhs=theta[:, p * QP + qc * 512 : p * QP + (qc + 1) * 512],
                        start=True,
                        stop=True,
                    )
                et = ep.tile([128, QP], BF16, tag="et")
                nc.scalar.activation(et, s_ps, AF.Exp, bias=expb[:])
                acc = acc_state["acc"]
                for qc in range(2):
                    nc.tensor.matmul(
                        acc[:, qc * 512 : (qc + 1) * 512],
                        lhsT=gT[:, c, :],
                        rhs=et[:, qc * 512 : (qc + 1) * 512],
                        start=first,
                        stop=last,
                    )
                ets = acc_state["ets"]
                ets.append(et)
                if len(ets) == 2:
                    pr = zp.tile([128, QP], BF16, tag="pair", bufs=4)
                    nc.vector.tensor_add(pr, ets[0], ets[1])
                    acc_state["ets"] = []
                    prs = acc_state["quads"][0]
                    prs.append(pr)
                    if len(prs) == 2:
                        qd = zp.tile([128, QP], BF16, tag="quad", bufs=10)
                        nc.vector.tensor_add(qd, prs[0], prs[1])
                        acc_state["quads"][0] = []
                        acc_state["quads"][1].append(qd)

            def pass_tail(p):
                """z reduce + normalize + out conv + residual + store."""
                quads = acc_state["quads"][1]
                assert len(quads) == 8
                acc_state["quads"][1] = []
                s0 = zp.tile([128, QP], BF16, tag="pair", bufs=4)
                s1 = zp.tile([128, QP], BF16, tag="pair", bufs=4)
                s2 = zp.tile([128, QP], BF16, tag="pair", bufs=4)
                s3 = zp.tile([128, QP], BF16, tag="pair", bufs=4)
                nc.vector.tensor_add(s0, quads[0], quads[1])
                nc.vector.tensor_add(s1, quads[2], quads[3])
                nc.vector.tensor_add(s2, quads[4], quads[5])
                nc.vector.tensor_add(s3, quads[6], quads[7])
                t0 = zp.tile([128, QP], BF16, tag="quad", bufs=10)
                t1 = zp.tile([128, QP], BF16, tag="quad", bufs=10)
                nc.vector.tensor_add(t0, s0, s1)
                nc.vector.tensor_add(t1, s2, s3)
                zfin = zp.tile([128, QP], BF16, tag="zfin", bufs=2)
                nc.vector.tensor_add(zfin, t0, t1)

                # z row sums -> bf16 row -> broadcast to 128 partitions on PE
                # -> reciprocal on the broadcast (full-width DVE op)
                for qc in range(2):
                    zrow = psc.tile([128, MPB], F32, tag="cv")
                    nc.tensor.matmul(
                        zrow[0:1, :],
                        lhsT=ones128b[:, 0:1],
                        rhs=zfin[:, qc * 512 : (qc + 1) * 512],
                        start=True,
                        stop=True,
                    )
                    nc.vector.tensor_copy(
                        zrowb[:, qc * 512 : (qc + 1) * 512], zrow[0:1, :]
                    )
                bps = pss.tile([128, QP], F32, tag="sc")
                for qc in range(2):
                    nc.tensor.matmul(
                        bps[:, qc * 512 : (qc + 1) * 512],
                        lhsT=ones1b[0:1, :],
                        rhs=zrowb[:, qc * 512 : (qc + 1) * 512],
                        start=True,
                        stop=True,
                    )
                invbc = op.tile([128, QP], F32, tag="bps", bufs=2)
                nc.vector.reciprocal(invbc, bps)
                wsb = pp.tile([128, NPASS, QP], F32R)
                nc.vector.tensor_mul(wsb[:, p, :], acc_state["acc"], invbc)
                for oc in range(CC):
                    for qc in range(2):
                        pso = psc.tile([128, MPB], F32, tag="cv")
                        nc.tensor.matmul(
                            pso,
                            lhsT=ww[:, oc, :],
                            rhs=wsb[:, p, qc * 512 : (qc + 1) * 512],
                            start=True,
                            stop=True,
                        )
                        osb = op.tile([128, MPB], F32, tag="ot")
                        nc.vector.tensor_add(
                            osb, pso,
                            center_sb[:, oc, p * QP + qc * 512 : p * QP + (qc + 1) * 512],
                        )
                        nc.sync.dma_start(
                            out=outd[oc][:, p * QP + qc * 512 : p * QP + (qc + 1) * 512],
                            in_=osb,
                        )

            # ---- m-loop: stream local feature, maxpool, exchange, conv,
            # and the local 16 chunks of pass 0 ----
            acc_state["acc"] = pacc.tile([128, QP], F32, tag="acc", name="acc")
            for m in range(NM):
                xloc = pk.tile([128, CC, MPB], BF16, tag="pk")
                for cc in range(CC):
                    ft = fp.tile([128, D, MPB], BF16, tag="ft")
                    nc.sync.dma_start(out=ft[:], in_=featd[cc, :, m])
                    t_a = mp.tile([128, MPB], BF16, tag="mp")
                    t_b = mp.tile([128, MPB], BF16, tag="mp")
                    t_c = mp.tile([128, MPB], BF16, tag="mp")
                    t_d = mp.tile([128, MPB], BF16, tag="mp")
                    nc.vector.tensor_max(t_a, ft[:, 0, :], ft[:, 1, :])
                    nc.vector.tensor_max(t_b, ft[:, 2, :], ft[:, 3, :])
                    nc.vector.tensor_max(t_c, ft[:, 4, :], ft[:, 5, :])
                    nc.vector.tensor_max(t_d, ft[:, 6, :], ft[:, 7, :])
                    nc.vector.tensor_max(t_a, t_a, t_b)
                    nc.vector.tensor_max(t_c, t_c, t_d)
                    nc.vector.tensor_max(t_a, t_a, t_c)
                    nc.vector.tensor_max(xloc[:, cc, :], t_a, ft[:, 8, :])
                nc.gpsimd.dma_start(out=pbd[m][:], in_=xloc[:, :, :])
                nc.gpsimd.collective_compute(
                    "AllGather", ALU.bypass, replica_groups=GROUPS,
                    ins=[pbd[m].ap().opt()], outs=[agd[m].ap().opt()],
                )
                for r in range(2):
                    for cc in range(CC):
                        nc.gpsimd.dma_start(
                            out=xall[:, cc, r * Q + m * MPB : r * Q + (m + 1) * MPB],
                            in_=agd[m][r][:, cc * MPB : (cc + 1) * MPB],
                        )
                conv_chunk(m * MPB, m)
                conv_chunk(Q + m * MPB, NM + m)
                cs = list(range(4 * m, 4 * m + 4)) + list(
                    range(16 + 4 * m, 16 + 4 * m + 4)
                )
                for c in cs:
                    att_chunk(0, c, first=(m == 0 and c == 0), last=(m == NM - 1 and c == cs[-1]))
            pass_tail(0)

            # ---- pass 1: everything resident ----
            acc_state["acc"] = pacc.tile([128, QP], F32, tag="acc", name="acc")
            for c in range(NCH):
                att_chunk(1, c, first=(c == 0), last=(c == NCH - 1))
            pass_tail(1)


def shard_inputs(inputs):
    f32 = np.float32
    bf16 = ml_dtypes.bfloat16
    feature = np.asarray(inputs["feature"], dtype=f32)
    w_theta = np.asarray(inputs["w_theta"], dtype=f32)
    w_phi = np.asarray(inputs["w_phi"], dtype=f32)
    w_g = np.asarray(inputs["w_g"], dtype=f32)
    w_w = np.asarray(inputs["w_w"], dtype=f32)

    # fold BN (inference) into the conv weights: y = W'x + b'
    sc_th = np.asarray(inputs["bn_theta_gamma"], f32) / np.sqrt(
        np.asarray(inputs["bn_theta_var"], f32) + EPS
    )
    b_th = np.asarray(inputs["bn_theta_beta"], f32) - np.asarray(
        inputs["bn_theta_mean"], f32
    ) * sc_th
    sc_ph = np.asarray(inputs["bn_phi_gamma"], f32) / np.sqrt(
        np.asarray(inputs["bn_phi_var"], f32) + EPS
    )
    b_ph = np.asarray(inputs["bn_phi_beta"], f32) - np.asarray(
        inputs["bn_phi_mean"], f32
    ) * sc_ph

    wth = np.ascontiguousarray((w_theta * sc_th[:, None]).T.reshape(2, 128, 128))
    wph = np.ascontiguousarray(
        (w_phi * sc_ph[:, None]).T.reshape(2, 128, 128)
    ).astype(bf16)
    wgT = np.ascontiguousarray(w_g.T.reshape(2, 128, 128)).astype(bf16)
    wwT = np.ascontiguousarray(w_w.T.reshape(128, 2, 128).transpose(1, 0, 2))

    in_maps = []
    for core in range(8):
        n, h = core // 2, core % 2
        fh = feature[n].reshape(2, 128, D, HW)[:, :, :, h * Q : (h + 1) * Q]
        feat = np.ascontiguousarray(
            fh.reshape(2, 128, D, NM, MPB).transpose(0, 1, 3, 2, 4).astype(bf16)
        )
        center = np.ascontiguousarray(
            feature[n][:, D // 2 + 1].reshape(256, HW)[:, h * Q : (h + 1) * Q]
            .reshape(2, 128, Q)
        )
        bnb = np.zeros((128, 2), dtype=f32)
        bnb[:, 0] = b_th
        bnb[:, 1] = b_ph
        in_maps.append(
            dict(feat=feat, center=center, wth=wth, wph=wph, wg=wgT,
                 ww=wwT, bnb=bnb)
        )
    return in_maps


def unshard_output(results, N=4):
    out = np.empty((N, 256, 64, 64), dtype=np.float32)
    flat = out.reshape(N, 256, HW)
    for core in range(8):
        n, qh = core // 2, core % 2
        flat[n][:, qh * Q : (qh + 1) * Q] = results[core]["out"].reshape(256, Q)
    return out


def make_nc():
    nc = bacc.Bacc("TRN2", target_bir_lowering=False, debug=False, num_devices=8)
    build(nc)
    nc.compile()
    return nc


# ---------------------------------------------------------------------------
# Public entrypoint: full (unsharded) inputs -> full output, running the Bass
# kernel SPMD across the 8 NeuronCores.
# ---------------------------------------------------------------------------
from concourse.bass_utils import run_bass_kernel_spmd

_NC_CACHE = []


def _get_nc():
    if not _NC_CACHE:
        _NC_CACHE.append(make_nc())
    return _NC_CACHE[0]


def kernel(**inputs):
    nc = _get_nc()
    in_maps = shard_inputs(inputs)
    res = run_bass_kernel_spmd(nc, in_maps, list(range(8)))
    return unshard_output(res.results)


# revision 19
# speedup vs baseline: 1.5858x; 1.2534x over previous
"""CSABlock Trainium2 kernel, plan C: act-engine-roofline pipeline.

Core = 2n + h (sample n, image half h). Each core:
  - streams its h-half of feature[n] as bf16 (9.4MB), maxpools over D on DVE
    (bf16 2x mode),
  - exchanges the pooled x chunks with its partner core via fine-grained
    per-512px ReduceScatter (mask trick keeps SPMD code uniform),
  - computes theta (f32r, BN folded into weights host-side) for its 2048
    queries and phi/gT (f32r/bf16) for all 4096 keys locally,
  - runs the 2048x4096 attention in two 1024-query passes; per 128-key chunk:
    scores (PE, f32r) -> exp (Act, the roofline engine: exp is its ONLY job)
    -> weighted accumulate (PE, bf16) -> z accumulation (DVE bf16 trees),
  - tail per pass: z column-sum + 1/z broadcast on PE (no DRAM round trips),
    out conv, residual, store.
"""

import numpy as np
import ml_dtypes

import concourse.bass as bass
import concourse.mybir as mybir
import concourse.tile as tile
from concourse import bacc

F32 = mybir.dt.float32
F32R = mybir.dt.float32r
BF16 = mybir.dt.bfloat16

C = 256
CC = 2            # channel blocks of 128
IC = 128
D = 9
HW = 4096
Q = 2048          # local query/key pixels per core
NM = 4            # streamed pixel chunks of the local half
MPB = Q // NM     # 512 px per chunk
NCH = HW // 128   # 32 key chunks of 128 px over the full image
QP = 1024         # queries per attention pass
NPASS = 2
EXP_BIAS = -30.0
EPS = 1e-5
GROUPS = [[0, 1], [2, 3], [4, 5], [6, 7]]

AF = mybir.ActivationFunctionType
ALU = mybir.AluOpType


def build(nc):
    featd = nc.dram_tensor("feat", [CC, 128, NM, D, MPB], BF16, kind="ExternalInput")
    centerd = nc.dram_tensor("center", [CC, 128, Q], F32R, kind="ExternalInput")
    wthd = nc.dram_tensor("wth", [CC, 128, 128], F32R, kind="ExternalInput")
    wphd = nc.dram_tensor("wph", [CC, 128, 128], BF16, kind="ExternalInput")
    wgd = nc.dram_tensor("wg", [CC, 128, 128], BF16, kind="ExternalInput")
    wwd = nc.dram_tensor("ww", [CC, 128, 128], F32R, kind="ExternalInput")
    bnbd = nc.dram_tensor("bnb", [128, 2], F32, kind="ExternalInput")
    outd = nc.dram_tensor("out", [CC, 128, Q], F32, kind="ExternalOutput")
    # per-m-chunk exchange staging: each core contributes its pooled x chunk
    # ([cc0 512px | cc1 512px] bf16); a 2-core AllGather returns both cores'
    # chunks in rank order. Both slots are read back into xall, so the global
    # key order is [core0's 2048 px | core1's 2048 px] on BOTH cores —
    # SPMD-uniform, and attention is key-permutation-invariant. Queries stay
    # local (center/theta are per-core data).
    pbd = [nc.dram_tensor(f"pb{m}", [128, 2 * MPB], BF16) for m in range(NM)]
    agd = [nc.dram_tensor(f"ag{m}", [2, 128, 2 * MPB], BF16) for m in range(NM)]
    # warm-up collective: the first collective of a NEFF pays a large mesh
    # setup latency; issue a tiny one immediately so the real exchanges are
    # not gated on it.
    wui = nc.dram_tensor("wui", [128, 16], BF16)
    wuo = nc.dram_tensor("wuo", [2, 128, 16], BF16)

    with tile.TileContext(nc) as tc:
        with (
            tc.tile_pool(name="persist", bufs=1) as pp,
            tc.tile_pool(name="fstream", bufs=3) as fp,
            tc.tile_pool(name="mp", bufs=6) as mp,
            tc.tile_pool(name="et", bufs=6) as ep,
            tc.tile_pool(name="zt", bufs=10) as zp,
            tc.tile_pool(name="pk", bufs=2) as pk,
            tc.tile_pool(name="ot", bufs=4) as op,
            tc.tile_pool(name="psacc", bufs=1, space="PSUM") as pacc,
            tc.tile_pool(name="pssc", bufs=2, space="PSUM") as pss,
            tc.tile_pool(name="pscv", bufs=2, space="PSUM") as psc,
        ):
            nc.gpsimd.collective_compute(
                "AllGather", ALU.bypass, replica_groups=GROUPS,
                ins=[wui.ap().opt()], outs=[wuo.ap().opt()],
            )

            # ---- small loads ----
            center_sb = pp.tile([128, CC, Q], F32R)
            wth = pp.tile([128, CC, 128], F32R)
            wph = pp.tile([128, CC, 128], BF16)
            wg = pp.tile([128, CC, 128], BF16)
            ww = pp.tile([128, CC, 128], F32R)
            bnb = pp.tile([128, 2], F32)
            for cc in range(CC):
                nc.sync.dma_start(out=center_sb[:, cc, :], in_=centerd[cc])
            nc.sync.dma_start(out=wth[:, 0, :], in_=wthd[0])
            nc.sync.dma_start(out=wth[:, 1, :], in_=wthd[1])
            nc.sync.dma_start(out=wph[:, 0, :], in_=wphd[0])
            nc.sync.dma_start(out=wph[:, 1, :], in_=wphd[1])
            nc.sync.dma_start(out=wg[:, 0, :], in_=wgd[0])
            nc.sync.dma_start(out=wg[:, 1, :], in_=wgd[1])
            nc.sync.dma_start(out=ww[:, 0, :], in_=wwd[0])
            nc.sync.dma_start(out=ww[:, 1, :], in_=wwd[1])
            nc.sync.dma_start(out=bnb[:], in_=bnbd[:])

            expb = pp.tile([128, 1], F32)
            nc.gpsimd.memset(expb, EXP_BIAS)
            ones1b = pp.tile([1, 128], BF16)
            nc.gpsimd.memset(ones1b, 1.0)
            ones128b = pp.tile([128, 1], BF16)
            nc.gpsimd.memset(ones128b, 1.0)

            # ---- persistent state ----
            theta = pp.tile([128, Q], F32R)
            xall = pp.tile([128, CC, HW], BF16)   # [ch, cc, px] pooled input
            phi = pp.tile([128, HW], F32R)
            gT = pp.tile([128, NCH, 128], BF16)   # [px-in-chunk, chunk, ch]
            zrowb = pp.tile([1, QP], BF16)

            # ---- theta = relu(wth' @ center + bth) ----
            for qh in range(2):
                ps_t = pss.tile([128, QP], F32, tag="sc")
                for cc in range(CC):
                    for qc in range(2):
                        nc.tensor.matmul(
                            ps_t[:, qc * 512 : (qc + 1) * 512],
                            lhsT=wth[:, cc, :],
                            rhs=center_sb[:, cc, qh * QP + qc * 512 : qh * QP + (qc + 1) * 512],
                            start=(cc == 0),
                            stop=(cc == 1),
                        )
                nc.vector.tensor_scalar(
                    theta[:, qh * QP : (qh + 1) * QP], ps_t,
                    bnb[:, 0:1], 0.0, ALU.add, ALU.max,
                )

            # ---- attention machinery ----
            acc_state = {"acc": None, "ets": [], "quads": [[], []]}

            def conv_chunk(off, chi):
                """phi/gT for 512 px starting at global px `off` (chunk chi*4)."""
                ps_p = psc.tile([128, MPB], F32, tag="cv")
                for cc in range(CC):
                    nc.tensor.matmul(
                        ps_p,
                        lhsT=wph[:, cc, :],
                        rhs=xall[:, cc, off : off + MPB],
                        start=(cc == 0),
                        stop=(cc == 1),
                    )
                nc.vector.tensor_scalar(
                    phi[:, off : off + MPB], ps_p, bnb[:, 1:2], 0.0, ALU.add, ALU.max,
                )
                ps_g = psc.tile([128, MPB], F32, tag="cv")
                for j in range(4):
                    for cc in range(CC):
                        nc.tensor.matmul(
                            ps_g[:, j * 128 : (j + 1) * 128],
                            lhsT=xall[:, cc, off + j * 128 : off + (j + 1) * 128],
                            rhs=wg[:, cc, :],
                            start=(cc == 0),
                            stop=(cc == 1),
                        )
                nc.vector.tensor_copy(gT[:, chi * 4 : chi * 4 + 4, :], ps_g)

            def att_chunk(p, c, first, last):
                """pass p, key chunk c: scores -> exp -> weighted + z tree."""
                s_ps = pss.tile([128, QP], F32, tag="sc")
                for qc in range(2):
                    nc.tensor.matmul(
                        s_ps[:, qc * 512 : (qc + 1) * 512],
                        lhsT=phi[:, c * 128 : (c + 1) * 128],
                        rhs=theta[:, p * QP + qc * 512 : p * QP + (qc + 1) * 512],
                        start=True,
                        stop=True,
                    )
                et = ep.tile([128, QP], BF16, tag="et")
                nc.scalar.activation(et, s_ps, AF.Exp, bias=expb[:])
                acc = acc_state["acc"]
                for qc in range(2):
                    nc.tensor.matmul(
                        acc[:, qc * 512 : (qc + 1) * 512],
                        lhsT=gT[:, c, :],
                        rhs=et[:, qc * 512 : (qc + 1) * 512],
                        start=first,
                        stop=last,
                    )
                ets = acc_state["ets"]
                ets.append(et)
                if len(ets) == 2:
                    pr = zp.tile([128, QP], BF16, tag="pair", bufs=4)
                    nc.vector.tensor_add(pr, ets[0], ets[1])
                    acc_state["ets"] = []
                    prs = acc_state["quads"][0]
                    prs.append(pr)
                    if len(prs) == 2:
                        qd = zp.tile([128, QP], BF16, tag="quad", bufs=10)
                        nc.vector.tensor_add(qd, prs[0], prs[1])
                        acc_state["quads"][0] = []
                        acc_state["quads"][1].append(qd)

            def pass_tail(p):
                """z reduce + normalize + out conv + residual + store."""
                quads = acc_state["quads"][1]
                assert len(quads) == 8
                acc_state["quads"][1] = []
                s0 = zp.tile([128, QP], BF16, tag="pair", bufs=4)
                s1 = zp.tile([128, QP], BF16, tag="pair", bufs=4)
                s2 = zp.tile([128, QP], BF16, tag="pair", bufs=4)
                s3 = zp.tile([128, QP], BF16, tag="pair", bufs=4)
                nc.vector.tensor_add(s0, quads[0], quads[1])
                nc.vector.tensor_add(s1, quads[2], quads[3])
                nc.vector.tensor_add(s2, quads[4], quads[5])
                nc.vector.tensor_add(s3, quads[6], quads[7])
                t0 = zp.tile([128, QP], BF16, tag="quad", bufs=10)
                t1 = zp.tile([128, QP], BF16, tag="quad", bufs=10)
                nc.vector.tensor_add(t0, s0, s1)
                nc.vector.tensor_add(t1, s2, s3)
                zfin = zp.tile([128, QP], BF16, tag="zfin", bufs=2)
                nc.vector.tensor_add(zfin, t0, t1)

                # z row sums -> bf16 row -> broadcast to 128 partitions on PE
                # -> reciprocal on the broadcast (full-width DVE op)
                for qc in range(2):
                    zrow = psc.tile([128, MPB], F32, tag="cv")
                    nc.tensor.matmul(
                        zrow[0:1, :],
                        lhsT=ones128b[:, 0:1],
                        rhs=zfin[:, qc * 512 : (qc + 1) * 512],
                        start=True,
                        stop=True,
                    )
                    nc.vector.tensor_copy(
                        zrowb[:, qc * 512 : (qc + 1) * 512], zrow[0:1, :]
                    )
                bps = pss.tile([128, QP], F32, tag="sc")
                for qc in range(2):
                    nc.tensor.matmul(
                        bps[:, qc * 512 : (qc + 1) * 512],
                        lhsT=ones1b[0:1, :],
                        rhs=zrowb[:, qc * 512 : (qc + 1) * 512],
                        start=True,
                        stop=True,
                    )
                invbc = op.tile([128, QP], F32, tag="bps", bufs=2)
                nc.vector.reciprocal(invbc, bps)
                wsb = pp.tile([128, NPASS, QP], F32R)
                nc.vector.tensor_mul(wsb[:, p, :], acc_state["acc"], invbc)
                for oc in range(CC):
                    for qc in range(2):
                        pso = psc.tile([128, MPB], F32, tag="cv")
                        nc.tensor.matmul(
                            pso,
                            lhsT=ww[:, oc, :],
                            rhs=wsb[:, p, qc * 512 : (qc + 1) * 512],
                            start=True,
                            stop=True,
                        )
                        osb = op.tile([128, MPB], F32, tag="ot")
                        nc.vector.tensor_add(
                            osb, pso,
                            center_sb[:, oc, p * QP + qc * 512 : p * QP + (qc + 1) * 512],
                        )
                        nc.sync.dma_start(
                            out=outd[oc][:, p * QP + qc * 512 : p * QP + (qc + 1) * 512],
                            in_=osb,
                        )

            # ---- m-loop: stream local feature, maxpool, exchange, conv,
            # and the local 16 chunks of pass 0 ----
            acc_state["acc"] = pacc.tile([128, QP], F32, tag="acc", name="acc")
            for m in range(NM):
                xloc = pk.tile([128, CC, MPB], BF16, tag="pk")
                for cc in range(CC):
                    ft = fp.tile([128, D, MPB], BF16, tag="ft")
                    nc.sync.dma_start(out=ft[:], in_=featd[cc, :, m])
                    t_a = mp.tile([128, MPB], BF16, tag="mp")
                    t_b = mp.tile([128, MPB], BF16, tag="mp")
                    t_c = mp.tile([128, MPB], BF16, tag="mp")
                    t_d = mp.tile([128, MPB], BF16, tag="mp")
                    nc.vector.tensor_max(t_a, ft[:, 0, :], ft[:, 1, :])
                    nc.vector.tensor_max(t_b, ft[:, 2, :], ft[:, 3, :])
                    nc.vector.tensor_max(t_c, ft[:, 4, :], ft[:, 5, :])
                    nc.vector.tensor_max(t_d, ft[:, 6, :], ft[:, 7, :])
                    nc.vector.tensor_max(t_a, t_a, t_b)
                    nc.vector.tensor_max(t_c, t_c, t_d)
                    nc.vector.tensor_max(t_a, t_a, t_c)
                    nc.vector.tensor_max(xloc[:, cc, :], t_a, ft[:, 8, :])
                nc.gpsimd.dma_start(out=pbd[m][:], in_=xloc[:, :, :])
                nc.gpsimd.collective_compute(
                    "AllGather", ALU.bypass, replica_groups=GROUPS,
                    ins=[pbd[m].ap().opt()], outs=[agd[m].ap().opt()],
                )
                for r in range(2):
                    for cc in range(CC):
                        nc.gpsimd.dma_start(
                            out=xall[:, cc, r * Q + m * MPB : r * Q + (m + 1) * MPB],
                            in_=agd[m][r][:, cc * MPB : (cc + 1) * MPB],
                        )
                conv_chunk(m * MPB, m)
                conv_chunk(Q + m * MPB, NM + m)
                cs = list(range(4 * m, 4 * m + 4)) + list(
                    range(16 + 4 * m, 16 + 4 * m + 4)
                )
                for c in cs:
                    att_chunk(0, c, first=(m == 0 and c == 0), last=(m == NM - 1 and c == cs[-1]))
            pass_tail(0)

            # ---- pass 1: everything resident ----
            acc_state["acc"] = pacc.tile([128, QP], F32, tag="acc", name="acc")
            for c in range(NCH):
                att_chunk(1, c, first=(c == 0), last=(c == NCH - 1))
            pass_tail(1)


def shard_inputs(inputs):
    f32 = np.float32
    bf16 = ml_dtypes.bfloat16
    feature = np.asarray(inputs["feature"], dtype=f32)
    w_theta = np.asarray(inputs["w_theta"], dtype=f32)
    w_phi = np.asarray(inputs["w_phi"], dtype=f32)
    w_g = np.asarray(inputs["w_g"], dtype=f32)
    w_w = np.asarray(inputs["w_w"], dtype=f32)

    # fold BN (inference) into the conv weights: y = W'x + b'
    sc_th = np.asarray(inputs["bn_theta_gamma"], f32) / np.sqrt(
        np.asarray(inputs["bn_theta_var"], f32) + EPS
    )
    b_th = np.asarray(inputs["bn_theta_beta"], f32) - np.asarray(
        inputs["bn_theta_mean"], f32
    ) * sc_th
    sc_ph = np.asarray(inputs["bn_phi_gamma"], f32) / np.sqrt(
        np.asarray(inputs["bn_phi_var"], f32) + EPS
    )
    b_ph = np.asarray(inputs["bn_phi_beta"], f32) - np.asarray(
        inputs["bn_phi_mean"], f32
    ) * sc_ph

    wth = np.ascontiguousarray((w_theta * sc_th[:, None]).T.reshape(2, 128, 128))
    wph = np.ascontiguousarray(
        (w_phi * sc_ph[:, None]).T.reshape(2, 128, 128)
    ).astype(bf16)
    wgT = np.ascontiguousarray(w_g.T.reshape(2, 128, 128)).astype(bf16)
    wwT = np.ascontiguousarray(w_w.T.reshape(128, 2, 128).transpose(1, 0, 2))

    in_maps = []
    for core in range(8):
        n, h = core // 2, core % 2
        fh = feature[n].reshape(2, 128, D, HW)[:, :, :, h * Q : (h + 1) * Q]
        feat = np.ascontiguousarray(
            fh.reshape(2, 128, D, NM, MPB).transpose(0, 1, 3, 2, 4).astype(bf16)
        )
        center = np.ascontiguousarray(
            feature[n][:, D // 2 + 1].reshape(256, HW)[:, h * Q : (h + 1) * Q]
            .reshape(2, 128, Q)
        )
        bnb = np.zeros((128, 2), dtype=f32)
        bnb[:, 0] = b_th
        bnb[:, 1] = b_ph
        in_maps.append(
            dict(feat=feat, center=center, wth=wth, wph=wph, wg=wgT,
                 ww=wwT, bnb=bnb)
        )
    return in_maps


def unshard_output(results, N=4):
    out = np.empty((N, 256, 64, 64), dtype=np.float32)
    flat = out.reshape(N, 256, HW)
    for core in range(8):
        n, qh = core // 2, core % 2
        flat[n][:, qh * Q : (qh + 1) * Q] = results[core]["out"].reshape(256, Q)
    return out


def make_nc():
    nc = bacc.Bacc("TRN2", target_bir_lowering=False, debug=False, num_devices=8)
    build(nc)
    nc.compile()
    return nc


# ---------------------------------------------------------------------------
# Public entrypoint: full (unsharded) inputs -> full output, running the Bass
# kernel SPMD across the 8 NeuronCores.
# ---------------------------------------------------------------------------
from concourse.bass_utils import run_bass_kernel_spmd

_NC_CACHE = []


def _get_nc():
    if not _NC_CACHE:
        _NC_CACHE.append(make_nc())
    return _NC_CACHE[0]


def kernel(**inputs):
    nc = _get_nc()
    in_maps = shard_inputs(inputs)
    res = run_bass_kernel_spmd(nc, in_maps, list(range(8)))
    return unshard_output(res.results)


# revision 20
# speedup vs baseline: 1.8162x; 1.1452x over previous
"""CSABlock Trainium2 kernel, plan D: act-roofline pipeline, local-first keys.

Core = 2n + h (sample n, image half h). Key px order per core is
[own 2048 | partner 2048] — attention is key-permutation-invariant, so each
core can start attention on its own half immediately while the partner half
is in flight.

  - stream own h-half of feature[n] as bf16, maxpool over D on DVE (bf16 2x),
  - exchange pooled x per 512-px chunk via mask-zeroed 2-core
    ReduceScatter(add) (SPMD-uniform partner delivery); a warm-up collective
    at t=0 absorbs the NEFF's first-collective mesh setup latency,
  - theta (f32r, BN folded host-side into weights) for the 2048 own queries;
    phi (f32r) / gT (bf16) for all 4096 keys computed locally,
  - attention in two 1024-query passes, interleaved so the exchange hides:
    p0-local, p1-local, p0-remote, p1-remote; per 128-key chunk:
    scores (PE f32r) -> exp (Act — its only job; the roofline) -> weighted
    accumulate (PE bf16, one PSUM acc per pass) -> z pair/quad/acc adds
    (DVE bf16, spread through the pass),
  - tails (both at the end, overlapped): z column-sum + 1/z broadcast on PE,
    reciprocal_approx_fast, normalize, out conv, residual, store.

PSUM: 2 pass accumulators (2 banks each) + a shared 2-deep [128,1024] ring
(theta/conv/scores/tails) = 8 banks exactly.
"""

import numpy as np
import ml_dtypes

import concourse.bass as bass
import concourse.mybir as mybir
import concourse.tile as tile
from concourse import bacc

F32 = mybir.dt.float32
F32R = mybir.dt.float32r
BF16 = mybir.dt.bfloat16

C = 256
CC = 2            # channel blocks of 128
IC = 128
D = 9
HW = 4096
Q = 2048          # local query/key pixels per core
NM = 4            # streamed pixel chunks of the local half
MPB = Q // NM     # 512 px per chunk
NCH = HW // 128   # 32 key chunks of 128 px over the full image
QP = 1024         # queries per attention pass
NPASS = 2
EXP_BIAS = -30.0
EPS = 1e-5
GROUPS = [[0, 1], [2, 3], [4, 5], [6, 7]]

AF = mybir.ActivationFunctionType
ALU = mybir.AluOpType


def build(nc):
    featd = nc.dram_tensor("feat", [CC, 128, NM, D, MPB], BF16, kind="ExternalInput")
    centerd = nc.dram_tensor("center", [CC, 128, Q], F32R, kind="ExternalInput")
    wthd = nc.dram_tensor("wth", [CC, 128, 128], F32R, kind="ExternalInput")
    wphd = nc.dram_tensor("wph", [CC, 128, 128], BF16, kind="ExternalInput")
    wgd = nc.dram_tensor("wg", [CC, 128, 128], BF16, kind="ExternalInput")
    wwd = nc.dram_tensor("ww", [CC, 128, 128], F32R, kind="ExternalInput")
    bnbd = nc.dram_tensor("bnb", [128, 4], F32, kind="ExternalInput")
    outd = nc.dram_tensor("out", [CC, 128, Q], F32, kind="ExternalOutput")
    # exchange staging: slot s carries this core's x masked for group rank s
    # (own slot zeroed); RS(add) delivers exactly the partner's x.
    pbd = [nc.dram_tensor(f"pb{m}", [2, 128, 2 * MPB], BF16) for m in range(NM)]
    rsd = [nc.dram_tensor(f"rs{m}", [128, 2 * MPB], BF16) for m in range(NM)]
    # warm-up collective: absorbs the NEFF's first-collective setup latency.
    wui = nc.dram_tensor("wui", [128, 16], BF16)
    wuo = nc.dram_tensor("wuo", [2, 128, 16], BF16)

    with tile.TileContext(nc) as tc:
        with (
            tc.tile_pool(name="persist", bufs=1) as pp,
            tc.tile_pool(name="fstream", bufs=3) as fp,
            tc.tile_pool(name="mp", bufs=6) as mp,
            tc.tile_pool(name="et", bufs=6) as ep,
            tc.tile_pool(name="zt", bufs=4) as zp,
            tc.tile_pool(name="pk", bufs=2) as pk,
            tc.tile_pool(name="ot", bufs=4) as op,
            tc.tile_pool(name="psacc", bufs=2, space="PSUM") as pacc,
            tc.tile_pool(name="pssc", bufs=2, space="PSUM") as pss,
        ):
            nc.gpsimd.collective_compute(
                "AllGather", ALU.bypass, replica_groups=GROUPS,
                ins=[wui.ap().opt()], outs=[wuo.ap().opt()],
            )

            # ---- small loads ----
            center_sb = pp.tile([128, CC, Q], F32R)
            wth = pp.tile([128, CC, 128], F32R)
            wph = pp.tile([128, CC, 128], BF16)
            wg = pp.tile([128, CC, 128], BF16)
            ww = pp.tile([128, CC, 128], F32R)
            bnb = pp.tile([128, 4], F32)
            for cc in range(CC):
                nc.sync.dma_start(out=center_sb[:, cc, :], in_=centerd[cc])
            nc.sync.dma_start(out=wth[:, 0, :], in_=wthd[0])
            nc.sync.dma_start(out=wth[:, 1, :], in_=wthd[1])
            nc.sync.dma_start(out=wph[:, 0, :], in_=wphd[0])
            nc.sync.dma_start(out=wph[:, 1, :], in_=wphd[1])
            nc.sync.dma_start(out=wg[:, 0, :], in_=wgd[0])
            nc.sync.dma_start(out=wg[:, 1, :], in_=wgd[1])
            nc.sync.dma_start(out=ww[:, 0, :], in_=wwd[0])
            nc.sync.dma_start(out=ww[:, 1, :], in_=wwd[1])
            nc.sync.dma_start(out=bnb[:], in_=bnbd[:])

            expb = pp.tile([128, 1], F32)
            nc.gpsimd.memset(expb, EXP_BIAS)
            ones1b = pp.tile([1, 128], BF16)
            nc.gpsimd.memset(ones1b, 1.0)
            ones128b = pp.tile([128, 1], BF16)
            nc.gpsimd.memset(ones128b, 1.0)

            # ---- persistent state ----
            theta = pp.tile([128, Q], F32R)
            xall = pp.tile([128, CC, HW], BF16)   # [ch, cc, px]; own | partner
            phi = pp.tile([128, HW], F32R)
            gT = pp.tile([128, NCH, 128], BF16)   # [px-in-chunk, chunk, ch]
            zrowb = pp.tile([1, NPASS, QP], BF16)
            wsb = pp.tile([128, NPASS, QP], F32R)

            # ---- theta = relu(wth' @ center + bth) ----
            for qh in range(2):
                ps_t = pss.tile([128, QP], F32, tag="sc")
                for cc in range(CC):
                    for qc in range(2):
                        nc.tensor.matmul(
                            ps_t[:, qc * 512 : (qc + 1) * 512],
                            lhsT=wth[:, cc, :],
                            rhs=center_sb[:, cc, qh * QP + qc * 512 : qh * QP + (qc + 1) * 512],
                            start=(cc == 0),
                            stop=(cc == 1),
                        )
                nc.vector.tensor_scalar(
                    theta[:, qh * QP : (qh + 1) * QP], ps_t,
                    bnb[:, 0:1], 0.0, ALU.add, ALU.max,
                )

            # ---- attention machinery ----
            st = {
                p: {"acc": None, "ets": [], "prs": [], "quads": [], "zacc": None}
                for p in range(NPASS)
            }

            def conv_chunk(off, chi):
                """phi/gT for 512 px starting at global px `off` (chunk chi*4)."""
                ps_p = pss.tile([128, QP], F32, tag="sc")
                for cc in range(CC):
                    nc.tensor.matmul(
                        ps_p[:, 0:MPB],
                        lhsT=wph[:, cc, :],
                        rhs=xall[:, cc, off : off + MPB],
                        start=(cc == 0),
                        stop=(cc == 1),
                    )
                nc.vector.tensor_scalar(
                    phi[:, off : off + MPB], ps_p[:, 0:MPB],
                    bnb[:, 1:2], 0.0, ALU.add, ALU.max,
                )
                ps_g = pss.tile([128, QP], F32, tag="sc")
                for j in range(4):
                    for cc in range(CC):
                        nc.tensor.matmul(
                            ps_g[:, j * 128 : (j + 1) * 128],
                            lhsT=xall[:, cc, off + j * 128 : off + (j + 1) * 128],
                            rhs=wg[:, cc, :],
                            start=(cc == 0),
                            stop=(cc == 1),
                        )
                nc.vector.tensor_copy(gT[:, chi * 4 : chi * 4 + 4, :], ps_g[:, 0:MPB])

            def att_chunk(p, c, first, last):
                """pass p, key chunk c: scores -> exp -> weighted + z adds."""
                s = st[p]
                s_ps = pss.tile([128, QP], F32, tag="sc")
                for qc in range(2):
                    nc.tensor.matmul(
                        s_ps[:, qc * 512 : (qc + 1) * 512],
                        lhsT=phi[:, c * 128 : (c + 1) * 128],
                        rhs=theta[:, p * QP + qc * 512 : p * QP + (qc + 1) * 512],
                        start=True,
                        stop=True,
                    )
                et = ep.tile([128, QP], BF16, tag="et")
                nc.scalar.activation(et, s_ps, AF.Exp, bias=expb[:])
                for qc in range(2):
                    nc.tensor.matmul(
                        s["acc"][:, qc * 512 : (qc + 1) * 512],
                        lhsT=gT[:, c, :],
                        rhs=et[:, qc * 512 : (qc + 1) * 512],
                        start=first,
                        stop=last,
                    )
                s["ets"].append(et)
                if len(s["ets"]) == 2:
                    pr = zp.tile([128, QP], BF16, tag="pair", bufs=4)
                    nc.vector.tensor_add(pr, s["ets"][0], s["ets"][1])
                    s["ets"] = []
                    s["prs"].append(pr)
                if len(s["prs"]) == 2:
                    qd = zp.tile([128, QP], BF16, tag="quad", bufs=4)
                    nc.vector.tensor_add(qd, s["prs"][0], s["prs"][1])
                    s["prs"] = []
                    s["quads"].append(qd)
                if len(s["quads"]) == 2:
                    q0, q1 = s["quads"]
                    s["quads"] = []
                    if s["zacc"] is None:
                        za = pp.tile([128, QP], BF16, name=f"zacc{p}")
                        s["zacc"] = za
                        nc.vector.tensor_add(za, q0, q1)
                    else:
                        tq = zp.tile([128, QP], BF16, tag="quad", bufs=4)
                        nc.vector.tensor_add(tq, q0, q1)
                        nc.vector.tensor_add(s["zacc"], s["zacc"], tq)

            def pass_tail(p):
                """z colsum + 1/z broadcast + normalize + out conv + store."""
                s = st[p]
                zrow = pss.tile([128, QP], F32, tag="sc")
                for qc in range(2):
                    nc.tensor.matmul(
                        zrow[0:1, qc * 512 : (qc + 1) * 512],
                        lhsT=ones128b[:, 0:1],
                        rhs=s["zacc"][:, qc * 512 : (qc + 1) * 512],
                        start=True,
                        stop=True,
                    )
                nc.vector.tensor_copy(zrowb[:, p, :], zrow[0:1, :])
                bps = pss.tile([128, QP], F32, tag="sc")
                for qc in range(2):
                    nc.tensor.matmul(
                        bps[:, qc * 512 : (qc + 1) * 512],
                        lhsT=ones1b[0:1, :],
                        rhs=zrowb[:, p, qc * 512 : (qc + 1) * 512],
                        start=True,
                        stop=True,
                    )
                invbc = op.tile([128, QP], F32, tag="bps", bufs=2)
                nc.vector.reciprocal_approx_fast(out=invbc, in_=bps)
                nc.vector.tensor_mul(wsb[:, p, :], s["acc"], invbc)
                for oc in range(CC):
                    pso = pss.tile([128, QP], F32, tag="sc")
                    for qc in range(2):
                        nc.tensor.matmul(
                            pso[:, qc * 512 : (qc + 1) * 512],
                            lhsT=ww[:, oc, :],
                            rhs=wsb[:, p, qc * 512 : (qc + 1) * 512],
                            start=True,
                            stop=True,
                        )
                    osb = op.tile([128, QP], F32, tag="ot")
                    nc.vector.tensor_add(
                        osb, pso, center_sb[:, oc, p * QP : (p + 1) * QP]
                    )
                    nc.sync.dma_start(
                        out=outd[oc][:, p * QP : (p + 1) * QP], in_=osb
                    )

            # ---- m-loop: stream local feature, maxpool, exchange, conv,
            # pass-0 local chunks ----
            st[0]["acc"] = pacc.tile([128, QP], F32, tag="acc", name="acc0")
            for m in range(NM):
                for cc in range(CC):
                    ft = fp.tile([128, D, MPB], BF16, tag="ft")
                    nc.sync.dma_start(out=ft[:], in_=featd[cc, :, m])
                    t_a = mp.tile([128, MPB], BF16, tag="mp")
                    t_b = mp.tile([128, MPB], BF16, tag="mp")
                    t_c = mp.tile([128, MPB], BF16, tag="mp")
                    t_d = mp.tile([128, MPB], BF16, tag="mp")
                    nc.vector.tensor_max(t_a, ft[:, 0, :], ft[:, 1, :])
                    nc.vector.tensor_max(t_b, ft[:, 2, :], ft[:, 3, :])
                    nc.vector.tensor_max(t_c, ft[:, 4, :], ft[:, 5, :])
                    nc.vector.tensor_max(t_d, ft[:, 6, :], ft[:, 7, :])
                    nc.vector.tensor_max(t_a, t_a, t_b)
                    nc.vector.tensor_max(t_c, t_c, t_d)
                    nc.vector.tensor_max(t_a, t_a, t_c)
                    nc.vector.tensor_max(
                        xall[:, cc, m * MPB : (m + 1) * MPB], t_a, ft[:, 8, :]
                    )
                # masked staging + RS exchange of this chunk
                pkm = pk.tile([128, 2, 2 * MPB], BF16, tag="pk")
                for sl in range(2):
                    for cc in range(CC):
                        nc.vector.tensor_scalar(
                            pkm[:, sl, cc * MPB : (cc + 1) * MPB],
                            xall[:, cc, m * MPB : (m + 1) * MPB],
                            bnb[:, 2 + sl : 3 + sl], None, ALU.mult,
                        )
                    nc.gpsimd.dma_start(out=pbd[m][sl], in_=pkm[:, sl, :])
                nc.gpsimd.collective_compute(
                    "ReduceScatter", ALU.add, replica_groups=GROUPS,
                    ins=[pbd[m].ap().opt()], outs=[rsd[m].ap().opt()],
                )
                for cc in range(CC):
                    nc.gpsimd.dma_start(
                        out=xall[:, cc, Q + m * MPB : Q + (m + 1) * MPB],
                        in_=rsd[m][:, cc * MPB : (cc + 1) * MPB],
                    )
                conv_chunk(m * MPB, m)
                for c in range(4 * m, 4 * m + 4):
                    att_chunk(0, c, first=(c == 0), last=False)

            # ---- pass-1 local chunks (all data resident) ----
            st[1]["acc"] = pacc.tile([128, QP], F32, tag="acc", name="acc1")
            for c in range(16):
                att_chunk(1, c, first=(c == 0), last=False)

            # ---- remote chunks (gated on the exchange) ----
            for rm in range(NM):
                conv_chunk(Q + rm * MPB, NM + rm)
                for c in range(16 + 4 * rm, 16 + 4 * rm + 4):
                    att_chunk(0, c, first=False, last=(c == NCH - 1))
            for c in range(16, NCH):
                att_chunk(1, c, first=False, last=(c == NCH - 1))

            # ---- tails (overlap each other at the end) ----
            pass_tail(0)
            pass_tail(1)


def shard_inputs(inputs):
    f32 = np.float32
    bf16 = ml_dtypes.bfloat16
    feature = np.asarray(inputs["feature"], dtype=f32)
    w_theta = np.asarray(inputs["w_theta"], dtype=f32)
    w_phi = np.asarray(inputs["w_phi"], dtype=f32)
    w_g = np.asarray(inputs["w_g"], dtype=f32)
    w_w = np.asarray(inputs["w_w"], dtype=f32)

    # fold BN (inference) into the conv weights: y = W'x + b'
    sc_th = np.asarray(inputs["bn_theta_gamma"], f32) / np.sqrt(
        np.asarray(inputs["bn_theta_var"], f32) + EPS
    )
    b_th = np.asarray(inputs["bn_theta_beta"], f32) - np.asarray(
        inputs["bn_theta_mean"], f32
    ) * sc_th
    sc_ph = np.asarray(inputs["bn_phi_gamma"], f32) / np.sqrt(
        np.asarray(inputs["bn_phi_var"], f32) + EPS
    )
    b_ph = np.asarray(inputs["bn_phi_beta"], f32) - np.asarray(
        inputs["bn_phi_mean"], f32
    ) * sc_ph

    wth = np.ascontiguousarray((w_theta * sc_th[:, None]).T.reshape(2, 128, 128))
    wph = np.ascontiguousarray(
        (w_phi * sc_ph[:, None]).T.reshape(2, 128, 128)
    ).astype(bf16)
    wgT = np.ascontiguousarray(w_g.T.reshape(2, 128, 128)).astype(bf16)
    wwT = np.ascontiguousarray(w_w.T.reshape(128, 2, 128).transpose(1, 0, 2))

    in_maps = []
    for core in range(8):
        n, h = core // 2, core % 2
        fh = feature[n].reshape(2, 128, D, HW)[:, :, :, h * Q : (h + 1) * Q]
        feat = np.ascontiguousarray(
            fh.reshape(2, 128, D, NM, MPB).transpose(0, 1, 3, 2, 4).astype(bf16)
        )
        center = np.ascontiguousarray(
            feature[n][:, D // 2 + 1].reshape(256, HW)[:, h * Q : (h + 1) * Q]
            .reshape(2, 128, Q)
        )
        bnb = np.zeros((128, 4), dtype=f32)
        bnb[:, 0] = b_th
        bnb[:, 1] = b_ph
        bnb[:, 2 + (1 - h)] = 1.0
        in_maps.append(
            dict(feat=feat, center=center, wth=wth, wph=wph, wg=wgT,
                 ww=wwT, bnb=bnb)
        )
    return in_maps


def unshard_output(results, N=4):
    out = np.empty((N, 256, 64, 64), dtype=np.float32)
    flat = out.reshape(N, 256, HW)
    for core in range(8):
        n, qh = core // 2, core % 2
        flat[n][:, qh * Q : (qh + 1) * Q] = results[core]["out"].reshape(256, Q)
    return out


def make_nc():
    nc = bacc.Bacc("TRN2", target_bir_lowering=False, debug=False, num_devices=8)
    build(nc)
    nc.compile()
    return nc


# ---------------------------------------------------------------------------
# Public entrypoint: full (unsharded) inputs -> full output, running the Bass
# kernel SPMD across the 8 NeuronCores.
# ---------------------------------------------------------------------------
from concourse.bass_utils import run_bass_kernel_spmd

_NC_CACHE = []


def _get_nc():
    if not _NC_CACHE:
        _NC_CACHE.append(make_nc())
    return _NC_CACHE[0]


def kernel(**inputs):
    nc = _get_nc()
    in_maps = shard_inputs(inputs)
    res = run_bass_kernel_spmd(nc, in_maps, list(range(8)))
    return unshard_output(res.results)


# revision 22
# speedup vs baseline: 1.8757x; 1.0328x over previous
"""CSABlock Trainium2 kernel, plan E: act-roofline pipeline, local-first keys,
software-pipelined PE stream.

Core = 2n + h (sample n, image half h). Key px order per core is
[own 2048 | partner 2048] — attention is key-permutation-invariant, so each
core starts attention on its own half immediately while the partner half is
exchanged.

  - own h-half of feature[n] streamed as bf16, maxpool over D on DVE (2x),
  - ONE mask-zeroed 2-core ReduceScatter(add) exchanges the full pooled half
    (per-collective overhead is ~15us, so fewer is better); a warm-up
    collective at t=0 absorbs the NEFF's first-collective mesh setup,
  - theta (f32r, BN folded host-side) for own queries — second query half
    deferred so the PE never stalls on the late center half,
  - attention in two 1024-query passes, segment order p0-local, p1-local,
    p0-remote, p1-remote; within a segment the scores matmuls run 2 chunks
    ahead of the weighted matmuls so the PE stream never waits on the exp
    and stays at full p-state,
  - z accumulated as bf16 pair/quad/running adds on DVE through the pass,
  - tails at the end: z column-sum + 1/z broadcast on PE,
    reciprocal_approx_fast, normalize, out conv, residual, store.

PSUM: 2 pass accumulators (2 banks each) + a shared 2-deep [128,1024] ring
(theta/conv/scores/tails) = 8 banks exactly.
"""

import numpy as np
import ml_dtypes

import concourse.bass as bass
import concourse.mybir as mybir
import concourse.tile as tile
from concourse import bacc

F32 = mybir.dt.float32
F32R = mybir.dt.float32r
BF16 = mybir.dt.bfloat16

C = 256
CC = 2            # channel blocks of 128
IC = 128
D = 9
HW = 4096
Q = 2048          # local query/key pixels per core
NM = 4            # streamed pixel chunks of the local half
MPB = Q // NM     # 512 px per chunk
NCH = HW // 128   # 32 key chunks of 128 px over the full image
QP = 1024         # queries per attention pass
NPASS = 2
EXP_BIAS = -30.0
EPS = 1e-5
GROUPS = [[0, 1], [2, 3], [4, 5], [6, 7]]

AF = mybir.ActivationFunctionType
ALU = mybir.AluOpType


def build(nc):
    featd = nc.dram_tensor("feat", [CC, 128, NM, D, MPB], BF16, kind="ExternalInput")
    centerd = nc.dram_tensor("center", [CC, 2, 128, QP], F32R, kind="ExternalInput")
    wthd = nc.dram_tensor("wth", [CC, 128, 128], F32R, kind="ExternalInput")
    wphd = nc.dram_tensor("wph", [CC, 128, 128], BF16, kind="ExternalInput")
    wgd = nc.dram_tensor("wg", [CC, 128, 128], BF16, kind="ExternalInput")
    wwd = nc.dram_tensor("ww", [CC, 128, 128], F32R, kind="ExternalInput")
    bnbd = nc.dram_tensor("bnb", [128, 4], F32, kind="ExternalInput")
    outd = nc.dram_tensor("out", [CC, 128, Q], F32, kind="ExternalOutput")
    # exchange staging: slot s carries this core's pooled x masked for group
    # rank s (own slot zeroed); one RS(add) delivers the partner's full half.
    pbd = nc.dram_tensor("pb", [2, 128, CC, Q], BF16)
    rsd = nc.dram_tensor("rs", [128, CC, Q], BF16)
    # warm-up collective: absorbs the NEFF's first-collective setup latency.
    wui = nc.dram_tensor("wui", [128, 16], BF16)
    wuo = nc.dram_tensor("wuo", [2, 128, 16], BF16)

    with tile.TileContext(nc) as tc:
        with (
            tc.tile_pool(name="persist", bufs=1) as pp,
            tc.tile_pool(name="fstream", bufs=3) as fp,
            tc.tile_pool(name="mp", bufs=6) as mp,
            tc.tile_pool(name="et", bufs=6) as ep,
            tc.tile_pool(name="zt", bufs=4) as zp,
            tc.tile_pool(name="pk", bufs=2) as pk,
            tc.tile_pool(name="ot", bufs=4) as op,
            tc.tile_pool(name="psacc", bufs=2, space="PSUM") as pacc,
            tc.tile_pool(name="pssc", bufs=2, space="PSUM") as pss,
        ):
            nc.gpsimd.collective_compute(
                "AllGather", ALU.bypass, replica_groups=GROUPS,
                ins=[wui.ap().opt()], outs=[wuo.ap().opt()],
            )

            # ---- small loads; center arrives per query-half so pass-0 work
            # is never gated on the second half ----
            center_sb = pp.tile([128, CC, Q], F32R)
            wth = pp.tile([128, CC, 128], F32R)
            wph = pp.tile([128, CC, 128], BF16)
            wg = pp.tile([128, CC, 128], BF16)
            ww = pp.tile([128, CC, 128], F32R)
            bnb = pp.tile([128, 4], F32)
            nc.sync.dma_start(out=wth[:, 0, :], in_=wthd[0])
            nc.sync.dma_start(out=wth[:, 1, :], in_=wthd[1])
            nc.sync.dma_start(out=wph[:, 0, :], in_=wphd[0])
            nc.sync.dma_start(out=wph[:, 1, :], in_=wphd[1])
            nc.sync.dma_start(out=wg[:, 0, :], in_=wgd[0])
            nc.sync.dma_start(out=wg[:, 1, :], in_=wgd[1])
            nc.sync.dma_start(out=ww[:, 0, :], in_=wwd[0])
            nc.sync.dma_start(out=ww[:, 1, :], in_=wwd[1])
            nc.sync.dma_start(out=bnb[:], in_=bnbd[:])
            for cc in range(CC):
                nc.sync.dma_start(
                    out=center_sb[:, cc, 0:QP], in_=centerd[cc, 0]
                )

            expb = pp.tile([128, 1], F32)
            nc.gpsimd.memset(expb, EXP_BIAS)
            ones1b = pp.tile([1, 128], BF16)
            nc.gpsimd.memset(ones1b, 1.0)
            ones128b = pp.tile([128, 1], BF16)
            nc.gpsimd.memset(ones128b, 1.0)

            # ---- persistent state ----
            theta = pp.tile([128, Q], F32R)
            xall = pp.tile([128, CC, HW], BF16)   # [ch, cc, px]; own | partner
            phi = pp.tile([128, HW], F32R)
            gT = pp.tile([128, NCH, 128], BF16)   # [px-in-chunk, chunk, ch]
            zrowb = pp.tile([1, NPASS, QP], BF16)
            wsb = pp.tile([128, NPASS, QP], F32R)

            def theta_half(qh):
                ps_t = pss.tile([128, QP], F32, tag="sc", name="ps_t")
                for cc in range(CC):
                    for qc in range(2):
                        nc.tensor.matmul(
                            ps_t[:, qc * 512 : (qc + 1) * 512],
                            lhsT=wth[:, cc, :],
                            rhs=center_sb[:, cc, qh * QP + qc * 512 : qh * QP + (qc + 1) * 512],
                            start=(cc == 0),
                            stop=(cc == 1),
                        )
                nc.vector.tensor_scalar(
                    theta[:, qh * QP : (qh + 1) * QP], ps_t,
                    bnb[:, 0:1], 0.0, ALU.add, ALU.max,
                )

            theta_half(0)

            # ---- attention machinery ----
            st = {
                p: {"acc": None, "ets": [], "prs": [], "quads": [], "zacc": None}
                for p in range(NPASS)
            }
            wq = []  # pending weighted matmuls: (p, c, et, first, last)

            def conv_chunk(off, chi):
                """phi/gT for 512 px starting at global px `off` (chunk chi*4)."""
                ps_p = pss.tile([128, QP], F32, tag="sc", name="ps_p")
                for cc in range(CC):
                    nc.tensor.matmul(
                        ps_p[:, 0:MPB],
                        lhsT=wph[:, cc, :],
                        rhs=xall[:, cc, off : off + MPB],
                        start=(cc == 0),
                        stop=(cc == 1),
                    )
                nc.vector.tensor_scalar(
                    phi[:, off : off + MPB], ps_p[:, 0:MPB],
                    bnb[:, 1:2], 0.0, ALU.add, ALU.max,
                )
                ps_g = pss.tile([128, QP], F32, tag="sc", name="ps_g")
                for j in range(4):
                    for cc in range(CC):
                        nc.tensor.matmul(
                            ps_g[:, j * 128 : (j + 1) * 128],
                            lhsT=xall[:, cc, off + j * 128 : off + (j + 1) * 128],
                            rhs=wg[:, cc, :],
                            start=(cc == 0),
                            stop=(cc == 1),
                        )
                nc.vector.tensor_copy(gT[:, chi * 4 : chi * 4 + 4, :], ps_g[:, 0:MPB])

            def emit_weighted(p, c, et, first, last):
                for qc in range(2):
                    nc.tensor.matmul(
                        st[p]["acc"][:, qc * 512 : (qc + 1) * 512],
                        lhsT=gT[:, c, :],
                        rhs=et[:, qc * 512 : (qc + 1) * 512],
                        start=first,
                        stop=last,
                    )

            def att_chunk(p, c, first, last):
                """scores -> exp (weighted deferred 2 chunks) + z adds."""
                s = st[p]
                s_ps = pss.tile([128, QP], F32, tag="sc", name="s_ps")
                for qc in range(2):
                    nc.tensor.matmul(
                        s_ps[:, qc * 512 : (qc + 1) * 512],
                        lhsT=phi[:, c * 128 : (c + 1) * 128],
                        rhs=theta[:, p * QP + qc * 512 : p * QP + (qc + 1) * 512],
                        start=True,
                        stop=True,
                    )
                et = ep.tile([128, QP], BF16, tag="et")
                nc.scalar.activation(et, s_ps, AF.Exp, bias=expb[:])
                wq.append((p, c, et, first, last))
                if len(wq) > 2:
                    emit_weighted(*wq.pop(0))
                s["ets"].append(et)
                if len(s["ets"]) == 2:
                    pr = zp.tile([128, QP], BF16, tag="pair", bufs=4)
                    nc.vector.tensor_add(pr, s["ets"][0], s["ets"][1])
                    s["ets"] = []
                    s["prs"].append(pr)
                if len(s["prs"]) == 2:
                    qd = zp.tile([128, QP], BF16, tag="quad", bufs=4)
                    nc.vector.tensor_add(qd, s["prs"][0], s["prs"][1])
                    s["prs"] = []
                    s["quads"].append(qd)
                if len(s["quads"]) == 2:
                    q0, q1 = s["quads"]
                    s["quads"] = []
                    if s["zacc"] is None:
                        za = pp.tile([128, QP], BF16, name=f"zacc{p}")
                        s["zacc"] = za
                        nc.vector.tensor_add(za, q0, q1)
                    else:
                        tq = zp.tile([128, QP], BF16, tag="quad", bufs=4)
                        nc.vector.tensor_add(tq, q0, q1)
                        nc.vector.tensor_add(s["zacc"], s["zacc"], tq)

            def flush_wq():
                while wq:
                    emit_weighted(*wq.pop(0))

            def pass_tail(p):
                """z colsum + 1/z broadcast + normalize + out conv + store."""
                s = st[p]
                zrow = pss.tile([128, QP], F32, tag="sc", name="zrow")
                for qc in range(2):
                    nc.tensor.matmul(
                        zrow[0:1, qc * 512 : (qc + 1) * 512],
                        lhsT=ones128b[:, 0:1],
                        rhs=s["zacc"][:, qc * 512 : (qc + 1) * 512],
                        start=True,
                        stop=True,
                    )
                nc.vector.tensor_copy(zrowb[:, p, :], zrow[0:1, :])
                bps = pss.tile([128, QP], F32, tag="sc", name="bps")
                for qc in range(2):
                    nc.tensor.matmul(
                        bps[:, qc * 512 : (qc + 1) * 512],
                        lhsT=ones1b[0:1, :],
                        rhs=zrowb[:, p, qc * 512 : (qc + 1) * 512],
                        start=True,
                        stop=True,
                    )
                invbc = op.tile([128, QP], F32, tag="bps", bufs=2)
                nc.vector.reciprocal_approx_fast(out=invbc, in_=bps)
                nc.vector.tensor_mul(wsb[:, p, :], s["acc"], invbc)
                for oc in range(CC):
                    pso = pss.tile([128, QP], F32, tag="sc", name="pso")
                    for qc in range(2):
                        nc.tensor.matmul(
                            pso[:, qc * 512 : (qc + 1) * 512],
                            lhsT=ww[:, oc, :],
                            rhs=wsb[:, p, qc * 512 : (qc + 1) * 512],
                            start=True,
                            stop=True,
                        )
                    osb = op.tile([128, QP], F32, tag="ot")
                    nc.vector.tensor_add(
                        osb, pso, center_sb[:, oc, p * QP : (p + 1) * QP]
                    )
                    nc.sync.dma_start(
                        out=outd[oc][:, p * QP : (p + 1) * QP], in_=osb
                    )

            # ---- m-loop: stream local feature, maxpool, conv, pass-0 local
            # chunks; masked staging trails the compute ----
            st[0]["acc"] = pacc.tile([128, QP], F32, tag="acc", name="acc0")
            for m in range(NM):
                for cc in range(CC):
                    ft = fp.tile([128, D, MPB], BF16, tag="ft")
                    nc.sync.dma_start(out=ft[:], in_=featd[cc, :, m])
                    t_a = mp.tile([128, MPB], BF16, tag="mp")
                    t_b = mp.tile([128, MPB], BF16, tag="mp")
                    t_c = mp.tile([128, MPB], BF16, tag="mp")
                    t_d = mp.tile([128, MPB], BF16, tag="mp")
                    nc.vector.tensor_max(t_a, ft[:, 0, :], ft[:, 1, :])
                    nc.vector.tensor_max(t_b, ft[:, 2, :], ft[:, 3, :])
                    nc.vector.tensor_max(t_c, ft[:, 4, :], ft[:, 5, :])
                    nc.vector.tensor_max(t_d, ft[:, 6, :], ft[:, 7, :])
                    nc.vector.tensor_max(t_a, t_a, t_b)
                    nc.vector.tensor_max(t_c, t_c, t_d)
                    nc.vector.tensor_max(t_a, t_a, t_c)
                    nc.vector.tensor_max(
                        xall[:, cc, m * MPB : (m + 1) * MPB], t_a, ft[:, 8, :]
                    )
                conv_chunk(m * MPB, m)
                for c in range(4 * m, 4 * m + 4):
                    att_chunk(0, c, first=(c == 0), last=False)
                # masked staging for the exchange (trails phi/attention)
                pkm = pk.tile([128, 2, 2 * MPB], BF16, tag="pk")
                for sl in range(2):
                    for cc in range(CC):
                        nc.vector.tensor_scalar(
                            pkm[:, sl, cc * MPB : (cc + 1) * MPB],
                            xall[:, cc, m * MPB : (m + 1) * MPB],
                            bnb[:, 2 + sl : 3 + sl], None, ALU.mult,
                        )
                    nc.gpsimd.dma_start(
                        out=pbd[sl, :, :, m * MPB : (m + 1) * MPB],
                        in_=pkm[:, sl, :].rearrange("p (cc q) -> p cc q", cc=CC),
                    )
            # one RS moves the whole masked half; readback into xall
            nc.gpsimd.collective_compute(
                "ReduceScatter", ALU.add, replica_groups=GROUPS,
                ins=[pbd.ap().opt()], outs=[rsd.ap().opt()],
            )
            for cc in range(CC):
                nc.gpsimd.dma_start(
                    out=xall[:, cc, Q : 2 * Q], in_=rsd[:, cc, :]
                )

            # ---- pass-1 local chunks (all local data resident) ----
            flush_wq()
            for cc in range(CC):
                nc.sync.dma_start(
                    out=center_sb[:, cc, QP : 2 * QP], in_=centerd[cc, 1]
                )
            theta_half(1)
            st[1]["acc"] = pacc.tile([128, QP], F32, tag="acc", name="acc1")
            for c in range(16):
                att_chunk(1, c, first=(c == 0), last=False)

            # ---- remote chunks (gated on the exchange) ----
            flush_wq()
            for rm in range(NM):
                conv_chunk(Q + rm * MPB, NM + rm)
                for c in range(16 + 4 * rm, 16 + 4 * rm + 4):
                    att_chunk(0, c, first=False, last=(c == NCH - 1))
            flush_wq()
            for c in range(16, NCH):
                att_chunk(1, c, first=False, last=(c == NCH - 1))
            flush_wq()

            # ---- tails (overlap each other at the end) ----
            pass_tail(0)
            pass_tail(1)


def shard_inputs(inputs):
    f32 = np.float32
    bf16 = ml_dtypes.bfloat16
    feature = np.asarray(inputs["feature"], dtype=f32)
    w_theta = np.asarray(inputs["w_theta"], dtype=f32)
    w_phi = np.asarray(inputs["w_phi"], dtype=f32)
    w_g = np.asarray(inputs["w_g"], dtype=f32)
    w_w = np.asarray(inputs["w_w"], dtype=f32)

    # fold BN (inference) into the conv weights: y = W'x + b'
    sc_th = np.asarray(inputs["bn_theta_gamma"], f32) / np.sqrt(
        np.asarray(inputs["bn_theta_var"], f32) + EPS
    )
    b_th = np.asarray(inputs["bn_theta_beta"], f32) - np.asarray(
        inputs["bn_theta_mean"], f32
    ) * sc_th
    sc_ph = np.asarray(inputs["bn_phi_gamma"], f32) / np.sqrt(
        np.asarray(inputs["bn_phi_var"], f32) + EPS
    )
    b_ph = np.asarray(inputs["bn_phi_beta"], f32) - np.asarray(
        inputs["bn_phi_mean"], f32
    ) * sc_ph

    wth = np.ascontiguousarray((w_theta * sc_th[:, None]).T.reshape(2, 128, 128))
    wph = np.ascontiguousarray(
        (w_phi * sc_ph[:, None]).T.reshape(2, 128, 128)
    ).astype(bf16)
    wgT = np.ascontiguousarray(w_g.T.reshape(2, 128, 128)).astype(bf16)
    wwT = np.ascontiguousarray(w_w.T.reshape(128, 2, 128).transpose(1, 0, 2))

    in_maps = []
    for core in range(8):
        n, h = core // 2, core % 2
        fh = feature[n].reshape(2, 128, D, HW)[:, :, :, h * Q : (h + 1) * Q]
        feat = np.ascontiguousarray(
            fh.reshape(2, 128, D, NM, MPB).transpose(0, 1, 3, 2, 4).astype(bf16)
        )
        center = np.ascontiguousarray(
            feature[n][:, D // 2 + 1].reshape(256, HW)[:, h * Q : (h + 1) * Q]
            .reshape(2, 128, 2, QP).transpose(0, 2, 1, 3)
        )
        bnb = np.zeros((128, 4), dtype=f32)
        bnb[:, 0] = b_th
        bnb[:, 1] = b_ph
        bnb[:, 2 + (1 - h)] = 1.0
        in_maps.append(
            dict(feat=feat, center=center, wth=wth, wph=wph, wg=wgT,
                 ww=wwT, bnb=bnb)
        )
    return in_maps


def unshard_output(results, N=4):
    out = np.empty((N, 256, 64, 64), dtype=np.float32)
    flat = out.reshape(N, 256, HW)
    for core in range(8):
        n, qh = core // 2, core % 2
        flat[n][:, qh * Q : (qh + 1) * Q] = results[core]["out"].reshape(256, Q)
    return out


def make_nc():
    nc = bacc.Bacc("TRN2", target_bir_lowering=False, debug=False, num_devices=8)
    build(nc)
    nc.compile()
    return nc


# ---------------------------------------------------------------------------
# Public entrypoint: full (unsharded) inputs -> full output, running the Bass
# kernel SPMD across the 8 NeuronCores.
# ---------------------------------------------------------------------------
from concourse.bass_utils import run_bass_kernel_spmd

_NC_CACHE = []


def _get_nc():
    if not _NC_CACHE:
        _NC_CACHE.append(make_nc())
    return _NC_CACHE[0]


def kernel(**inputs):
    nc = _get_nc()
    in_maps = shard_inputs(inputs)
    res = run_bass_kernel_spmd(nc, in_maps, list(range(8)))
    return unshard_output(res.results)


# revision 24
# speedup vs baseline: 1.9844x; 1.0580x over previous
"""CSABlock Trainium2 kernel, plan E: act-roofline pipeline, local-first keys,
software-pipelined PE stream.

Core = 2n + h (sample n, image half h). Key px order per core is
[own 2048 | partner 2048] — attention is key-permutation-invariant, so each
core starts attention on its own half immediately while the partner half is
exchanged.

  - own h-half of feature[n] streamed as bf16, maxpool over D on DVE (2x),
  - ONE mask-zeroed 2-core ReduceScatter(add) exchanges the full pooled half
    (per-collective overhead is ~15us, so fewer is better); a warm-up
    collective at t=0 absorbs the NEFF's first-collective mesh setup,
  - theta (f32r, BN folded host-side) for own queries — second query half
    deferred so the PE never stalls on the late center half,
  - attention in two 1024-query passes, segment order p0-local, p1-local,
    p0-remote, p1-remote; within a segment the scores matmuls run 2 chunks
    ahead of the weighted matmuls so the PE stream never waits on the exp
    and stays at full p-state,
  - z accumulated as bf16 pair/quad/running adds on DVE through the pass,
  - tails at the end: z column-sum + 1/z broadcast on PE,
    reciprocal_approx_fast, normalize, out conv, residual, store.

PSUM: 2 pass accumulators (2 banks each) + a shared 2-deep [128,1024] ring
(theta/conv/scores/tails) = 8 banks exactly.
"""

import numpy as np
import ml_dtypes

import concourse.bass as bass
import concourse.mybir as mybir
import concourse.tile as tile
from concourse import bacc

F32 = mybir.dt.float32
F32R = mybir.dt.float32r
BF16 = mybir.dt.bfloat16

C = 256
CC = 2            # channel blocks of 128
IC = 128
D = 9
HW = 4096
Q = 2048          # local query/key pixels per core
NM = 4            # streamed pixel chunks of the local half
MPB = Q // NM     # 512 px per chunk
NCH = HW // 128   # 32 key chunks of 128 px over the full image
QP = 1024         # queries per attention pass
NPASS = 2
EXP_BIAS = -30.0
EPS = 1e-5
GROUPS = [[0, 1], [2, 3], [4, 5], [6, 7]]

AF = mybir.ActivationFunctionType
ALU = mybir.AluOpType


def build(nc):
    featd = nc.dram_tensor("feat", [CC, 128, NM, D, MPB], BF16, kind="ExternalInput")
    centerd = nc.dram_tensor("center", [CC, 2, 128, QP], F32R, kind="ExternalInput")
    wthd = nc.dram_tensor("wth", [CC, 128, 128], F32R, kind="ExternalInput")
    wphd = nc.dram_tensor("wph", [CC, 128, 128], BF16, kind="ExternalInput")
    wgd = nc.dram_tensor("wg", [CC, 128, 128], BF16, kind="ExternalInput")
    wwd = nc.dram_tensor("ww", [CC, 128, 128], F32R, kind="ExternalInput")
    bnbd = nc.dram_tensor("bnb", [128, 4], F32, kind="ExternalInput")
    outd = nc.dram_tensor("out", [CC, 128, Q], F32, kind="ExternalOutput")
    # exchange staging: slot s carries this core's pooled x masked for group
    # rank s (own slot zeroed); one RS(add) delivers the partner's full half.
    pbd = [nc.dram_tensor(f"pb{h}", [2, 128, CC, Q // 2], BF16) for h in range(2)]
    rsd = [nc.dram_tensor(f"rs{h}", [128, CC, Q // 2], BF16) for h in range(2)]
    # warm-up collective: absorbs the NEFF's first-collective setup latency.
    wui = nc.dram_tensor("wui", [128, 16], BF16)
    wuo = nc.dram_tensor("wuo", [2, 128, 16], BF16)

    with tile.TileContext(nc) as tc:
        with (
            tc.tile_pool(name="persist", bufs=1) as pp,
            tc.tile_pool(name="fstream", bufs=3) as fp,
            tc.tile_pool(name="mp", bufs=6) as mp,
            tc.tile_pool(name="et", bufs=8) as ep,
            tc.tile_pool(name="zt", bufs=4) as zp,
            tc.tile_pool(name="pk", bufs=2) as pk,
            tc.tile_pool(name="ot", bufs=4) as op,
            tc.tile_pool(name="psacc", bufs=2, space="PSUM") as pacc,
            tc.tile_pool(name="pssc", bufs=2, space="PSUM") as pss,
        ):
            nc.gpsimd.collective_compute(
                "AllGather", ALU.bypass, replica_groups=GROUPS,
                ins=[wui.ap().opt()], outs=[wuo.ap().opt()],
            )

            # ---- small loads; center arrives per query-half so pass-0 work
            # is never gated on the second half ----
            center_sb = pp.tile([128, CC, Q], F32R)
            wth = pp.tile([128, CC, 128], F32R)
            wph = pp.tile([128, CC, 128], BF16)
            wg = pp.tile([128, CC, 128], BF16)
            ww = pp.tile([128, CC, 128], F32R)
            bnb = pp.tile([128, 4], F32)
            nc.sync.dma_start(out=wth[:, 0, :], in_=wthd[0])
            nc.sync.dma_start(out=wth[:, 1, :], in_=wthd[1])
            nc.sync.dma_start(out=wph[:, 0, :], in_=wphd[0])
            nc.sync.dma_start(out=wph[:, 1, :], in_=wphd[1])
            nc.sync.dma_start(out=wg[:, 0, :], in_=wgd[0])
            nc.sync.dma_start(out=wg[:, 1, :], in_=wgd[1])
            nc.sync.dma_start(out=ww[:, 0, :], in_=wwd[0])
            nc.sync.dma_start(out=ww[:, 1, :], in_=wwd[1])
            nc.sync.dma_start(out=bnb[:], in_=bnbd[:])
            for cc in range(CC):
                nc.sync.dma_start(
                    out=center_sb[:, cc, 0:QP], in_=centerd[cc, 0]
                )

            expb = pp.tile([128, 1], F32)
            nc.gpsimd.memset(expb, EXP_BIAS)
            ones1b = pp.tile([1, 128], BF16)
            nc.gpsimd.memset(ones1b, 1.0)
            ones128b = pp.tile([128, 1], BF16)
            nc.gpsimd.memset(ones128b, 1.0)

            # ---- persistent state ----
            theta = pp.tile([128, Q], F32R)
            xall = pp.tile([128, CC, HW], BF16)   # [ch, cc, px]; own | partner
            phi = pp.tile([128, HW], F32R)
            gT = pp.tile([128, NCH, 128], BF16)   # [px-in-chunk, chunk, ch]
            zrowb = pp.tile([1, NPASS, QP], BF16)
            wsb = pp.tile([128, NPASS, QP], F32R)

            def theta_half(qh):
                ps_t = pss.tile([128, QP], F32, tag="sc", name="ps_t")
                for cc in range(CC):
                    for qc in range(2):
                        nc.tensor.matmul(
                            ps_t[:, qc * 512 : (qc + 1) * 512],
                            lhsT=wth[:, cc, :],
                            rhs=center_sb[:, cc, qh * QP + qc * 512 : qh * QP + (qc + 1) * 512],
                            start=(cc == 0),
                            stop=(cc == 1),
                        )
                nc.vector.tensor_scalar(
                    theta[:, qh * QP : (qh + 1) * QP], ps_t,
                    bnb[:, 0:1], 0.0, ALU.add, ALU.max,
                )

            theta_half(0)

            # ---- attention machinery ----
            st = {
                p: {"acc": None, "ets": [], "prs": [], "quads": [], "zacc": None}
                for p in range(NPASS)
            }
            wq = []  # pending weighted matmuls: (p, c, et, first, last)

            def conv_chunk(off, chi):
                """phi/gT for 512 px starting at global px `off` (chunk chi*4)."""
                ps_p = pss.tile([128, QP], F32, tag="sc", name="ps_p")
                for cc in range(CC):
                    nc.tensor.matmul(
                        ps_p[:, 0:MPB],
                        lhsT=wph[:, cc, :],
                        rhs=xall[:, cc, off : off + MPB],
                        start=(cc == 0),
                        stop=(cc == 1),
                    )
                nc.vector.tensor_scalar(
                    phi[:, off : off + MPB], ps_p[:, 0:MPB],
                    bnb[:, 1:2], 0.0, ALU.add, ALU.max,
                )
                ps_g = pss.tile([128, QP], F32, tag="sc", name="ps_g")
                for j in range(4):
                    for cc in range(CC):
                        nc.tensor.matmul(
                            ps_g[:, j * 128 : (j + 1) * 128],
                            lhsT=xall[:, cc, off + j * 128 : off + (j + 1) * 128],
                            rhs=wg[:, cc, :],
                            start=(cc == 0),
                            stop=(cc == 1),
                        )
                nc.vector.tensor_copy(gT[:, chi * 4 : chi * 4 + 4, :], ps_g[:, 0:MPB])

            def emit_weighted(p, c, et, first, last):
                for qc in range(2):
                    nc.tensor.matmul(
                        st[p]["acc"][:, qc * 512 : (qc + 1) * 512],
                        lhsT=gT[:, c, :],
                        rhs=et[:, qc * 512 : (qc + 1) * 512],
                        start=first,
                        stop=last,
                    )

            def att_chunk(p, c, first, last):
                """scores -> exp (weighted deferred 2 chunks) + z adds."""
                s = st[p]
                s_ps = pss.tile([128, QP], F32, tag="sc", name="s_ps")
                for qc in range(2):
                    nc.tensor.matmul(
                        s_ps[:, qc * 512 : (qc + 1) * 512],
                        lhsT=phi[:, c * 128 : (c + 1) * 128],
                        rhs=theta[:, p * QP + qc * 512 : p * QP + (qc + 1) * 512],
                        start=True,
                        stop=True,
                    )
                et = ep.tile([128, QP], BF16, tag="et")
                nc.scalar.activation(et, s_ps, AF.Exp, bias=expb[:])
                wq.append((p, c, et, first, last))
                if len(wq) > 2:
                    emit_weighted(*wq.pop(0))
                s["ets"].append(et)
                if len(s["ets"]) == 2:
                    pr = zp.tile([128, QP], BF16, tag="pair", bufs=4)
                    nc.vector.tensor_add(pr, s["ets"][0], s["ets"][1])
                    s["ets"] = []
                    s["prs"].append(pr)
                if len(s["prs"]) == 2:
                    qd = zp.tile([128, QP], BF16, tag="quad", bufs=4)
                    nc.vector.tensor_add(qd, s["prs"][0], s["prs"][1])
                    s["prs"] = []
                    s["quads"].append(qd)
                if len(s["quads"]) == 2:
                    q0, q1 = s["quads"]
                    s["quads"] = []
                    if s["zacc"] is None:
                        za = pp.tile([128, QP], BF16, name=f"zacc{p}")
                        s["zacc"] = za
                        nc.vector.tensor_add(za, q0, q1)
                    else:
                        tq = zp.tile([128, QP], BF16, tag="quad", bufs=4)
                        nc.vector.tensor_add(tq, q0, q1)
                        nc.vector.tensor_add(s["zacc"], s["zacc"], tq)

            def flush_wq():
                while wq:
                    emit_weighted(*wq.pop(0))

            def pass_tail(p):
                """z colsum + 1/z broadcast + normalize + out conv + store."""
                s = st[p]
                zrow = pss.tile([128, QP], F32, tag="sc", name="zrow")
                for qc in range(2):
                    nc.tensor.matmul(
                        zrow[0:1, qc * 512 : (qc + 1) * 512],
                        lhsT=ones128b[:, 0:1],
                        rhs=s["zacc"][:, qc * 512 : (qc + 1) * 512],
                        start=True,
                        stop=True,
                    )
                nc.vector.tensor_copy(zrowb[:, p, :], zrow[0:1, :])
                bps = pss.tile([128, QP], F32, tag="sc", name="bps")
                for qc in range(2):
                    nc.tensor.matmul(
                        bps[:, qc * 512 : (qc + 1) * 512],
                        lhsT=ones1b[0:1, :],
                        rhs=zrowb[:, p, qc * 512 : (qc + 1) * 512],
                        start=True,
                        stop=True,
                    )
                invbc = op.tile([128, QP], F32, tag="bps", bufs=2)
                nc.vector.reciprocal_approx_fast(out=invbc, in_=bps)
                nc.vector.tensor_mul(wsb[:, p, :], s["acc"], invbc)
                for oc in range(CC):
                    pso = pss.tile([128, QP], F32, tag="sc", name="pso")
                    for qc in range(2):
                        nc.tensor.matmul(
                            pso[:, qc * 512 : (qc + 1) * 512],
                            lhsT=ww[:, oc, :],
                            rhs=wsb[:, p, qc * 512 : (qc + 1) * 512],
                            start=True,
                            stop=True,
                        )
                    osb = op.tile([128, QP], F32, tag="ot")
                    nc.vector.tensor_add(
                        osb, pso, center_sb[:, oc, p * QP : (p + 1) * QP]
                    )
                    nc.sync.dma_start(
                        out=outd[oc][:, p * QP : (p + 1) * QP], in_=osb
                    )

            # ---- m-loop: stream local feature, maxpool, conv, pass-0 local
            # chunks; masked staging trails the compute ----
            st[0]["acc"] = pacc.tile([128, QP], F32, tag="acc", name="acc0")
            for m in range(NM):
                for cc in range(CC):
                    ft = fp.tile([128, D, MPB], BF16, tag="ft")
                    nc.sync.dma_start(out=ft[:], in_=featd[cc, :, m])
                    t_a = mp.tile([128, MPB], BF16, tag="mp")
                    t_b = mp.tile([128, MPB], BF16, tag="mp")
                    t_c = mp.tile([128, MPB], BF16, tag="mp")
                    t_d = mp.tile([128, MPB], BF16, tag="mp")
                    nc.vector.tensor_max(t_a, ft[:, 0, :], ft[:, 1, :])
                    nc.vector.tensor_max(t_b, ft[:, 2, :], ft[:, 3, :])
                    nc.vector.tensor_max(t_c, ft[:, 4, :], ft[:, 5, :])
                    nc.vector.tensor_max(t_d, ft[:, 6, :], ft[:, 7, :])
                    nc.vector.tensor_max(t_a, t_a, t_b)
                    nc.vector.tensor_max(t_c, t_c, t_d)
                    nc.vector.tensor_max(t_a, t_a, t_c)
                    nc.vector.tensor_max(
                        xall[:, cc, m * MPB : (m + 1) * MPB], t_a, ft[:, 8, :]
                    )
                conv_chunk(m * MPB, m)
                for c in range(4 * m, 4 * m + 4):
                    att_chunk(0, c, first=(c == 0), last=False)
                # masked staging for the exchange (on the scalar engine,
                # which has slack here; Copy+scale needs no act table)
                pkm = pk.tile([128, 2, 2 * MPB], BF16, tag="pk")
                for sl in range(2):
                    for cc in range(CC):
                        nc.scalar.mul(
                            pkm[:, sl, cc * MPB : (cc + 1) * MPB],
                            xall[:, cc, m * MPB : (m + 1) * MPB],
                            bnb[:, 2 + sl : 3 + sl],
                        )
                    nc.gpsimd.dma_start(
                        out=pbd[m // 2][sl, :, :, (m % 2) * MPB : (m % 2 + 1) * MPB],
                        in_=pkm[:, sl, :].rearrange("p (cc q) -> p cc q", cc=CC),
                    )
                if m % 2 == 1:
                    h = m // 2
                    nc.gpsimd.collective_compute(
                        "ReduceScatter", ALU.add, replica_groups=GROUPS,
                        ins=[pbd[h].ap().opt()], outs=[rsd[h].ap().opt()],
                    )
                    for cc in range(CC):
                        nc.gpsimd.dma_start(
                            out=xall[:, cc, Q + h * QP : Q + (h + 1) * QP],
                            in_=rsd[h][:, cc, :],
                        )

            # ---- pass-1 local chunks (all local data resident) ----
            flush_wq()
            for cc in range(CC):
                nc.sync.dma_start(
                    out=center_sb[:, cc, QP : 2 * QP], in_=centerd[cc, 1]
                )
            theta_half(1)
            st[1]["acc"] = pacc.tile([128, QP], F32, tag="acc", name="acc1")
            for c in range(16):
                att_chunk(1, c, first=(c == 0), last=False)

            # ---- remote chunks (gated on the exchange) ----
            flush_wq()
            for rm in range(NM):
                conv_chunk(Q + rm * MPB, NM + rm)
                for c in range(16 + 4 * rm, 16 + 4 * rm + 4):
                    att_chunk(0, c, first=False, last=(c == NCH - 1))
            flush_wq()
            for c in range(16, NCH):
                att_chunk(1, c, first=False, last=(c == NCH - 1))
            flush_wq()

            # ---- tails (overlap each other at the end) ----
            pass_tail(0)
            pass_tail(1)


def shard_inputs(inputs):
    f32 = np.float32
    bf16 = ml_dtypes.bfloat16
    feature = np.asarray(inputs["feature"], dtype=f32)
    w_theta = np.asarray(inputs["w_theta"], dtype=f32)
    w_phi = np.asarray(inputs["w_phi"], dtype=f32)
    w_g = np.asarray(inputs["w_g"], dtype=f32)
    w_w = np.asarray(inputs["w_w"], dtype=f32)

    # fold BN (inference) into the conv weights: y = W'x + b'
    sc_th = np.asarray(inputs["bn_theta_gamma"], f32) / np.sqrt(
        np.asarray(inputs["bn_theta_var"], f32) + EPS
    )
    b_th = np.asarray(inputs["bn_theta_beta"], f32) - np.asarray(
        inputs["bn_theta_mean"], f32
    ) * sc_th
    sc_ph = np.asarray(inputs["bn_phi_gamma"], f32) / np.sqrt(
        np.asarray(inputs["bn_phi_var"], f32) + EPS
    )
    b_ph = np.asarray(inputs["bn_phi_beta"], f32) - np.asarray(
        inputs["bn_phi_mean"], f32
    ) * sc_ph

    wth = np.ascontiguousarray((w_theta * sc_th[:, None]).T.reshape(2, 128, 128))
    wph = np.ascontiguousarray(
        (w_phi * sc_ph[:, None]).T.reshape(2, 128, 128)
    ).astype(bf16)
    wgT = np.ascontiguousarray(w_g.T.reshape(2, 128, 128)).astype(bf16)
    wwT = np.ascontiguousarray(w_w.T.reshape(128, 2, 128).transpose(1, 0, 2))

    in_maps = []
    for core in range(8):
        n, h = core // 2, core % 2
        fh = feature[n].reshape(2, 128, D, HW)[:, :, :, h * Q : (h + 1) * Q]
        feat = np.ascontiguousarray(
            fh.reshape(2, 128, D, NM, MPB).transpose(0, 1, 3, 2, 4).astype(bf16)
        )
        center = np.ascontiguousarray(
            feature[n][:, D // 2 + 1].reshape(256, HW)[:, h * Q : (h + 1) * Q]
            .reshape(2, 128, 2, QP).transpose(0, 2, 1, 3)
        )
        bnb = np.zeros((128, 4), dtype=f32)
        bnb[:, 0] = b_th
        bnb[:, 1] = b_ph
        bnb[:, 2 + (1 - h)] = 1.0
        in_maps.append(
            dict(feat=feat, center=center, wth=wth, wph=wph, wg=wgT,
                 ww=wwT, bnb=bnb)
        )
    return in_maps


def unshard_output(results, N=4):
    out = np.empty((N, 256, 64, 64), dtype=np.float32)
    flat = out.reshape(N, 256, HW)
    for core in range(8):
        n, qh = core // 2, core % 2
        flat[n][:, qh * Q : (qh + 1) * Q] = results[core]["out"].reshape(256, Q)
    return out


def make_nc():
    nc = bacc.Bacc("TRN2", target_bir_lowering=False, debug=False, num_devices=8)
    build(nc)
    nc.compile()
    return nc


# ---------------------------------------------------------------------------
# Public entrypoint: full (unsharded) inputs -> full output, running the Bass
# kernel SPMD across the 8 NeuronCores.
# ---------------------------------------------------------------------------
from concourse.bass_utils import run_bass_kernel_spmd

_NC_CACHE = []


def _get_nc():
    if not _NC_CACHE:
        _NC_CACHE.append(make_nc())
    return _NC_CACHE[0]


def kernel(**inputs):
    nc = _get_nc()
    in_maps = shard_inputs(inputs)
    res = run_bass_kernel_spmd(nc, in_maps, list(range(8)))
    return unshard_output(res.results)
